# revision 6
# baseline (speedup 1.0000x reference)
"""Multi-headed attention (B=8, S=1024, D=1024, H=16) on 8 TRN2 NeuronCores.

Strategy: pure data parallelism over the batch — core b computes batch element b
end-to-end (no collectives). All matmuls in bf16 (fp32 PSUM accumulation).

Per-core dataflow (everything "T" is feature-major [D, S]):
  inputs (host-pretransposed, bf16): qT, keyT, valT, wkT, wvT, woT
  1. K_T[d_out, s]  = matmul(lhsT=wkT, rhs=keyT) + bk      (bias per-partition)
  2. V[s, d_out]    = matmul(lhsT=valT, rhs=wvT) + bv      -> packed [s, h, 65]
                      with a ones column per head (gives softmax denominators
                      for free inside the p@v matmul)
  3. per head h: scoresT[k, q] = matmul(lhsT=K_T_h[64,128], rhs=qT_h[64,512])
                 pT = exp(scoresT / 8)  (ACT; max-subtraction skipped — scores
                 are provably small for this problem)
  4. xT_h[65, q] accum = matmul(lhsT=[V_h | 1][128,65], rhs=pT[128,512]);
     row 64 = softmax denominator; normalize rows 0..63 by its reciprocal
     (partition-broadcast via DMA)
  5. O[s, d_out] = matmul(lhsT=xT, rhs=woT) + bo -> DMA out (f32)
"""

import numpy as np
import ml_dtypes

import concourse.bass as bass
import concourse.bacc as bacc
import concourse.mybir as mybir
import concourse.tile as tile
from contextlib import ExitStack

B, S, D, H = 8, 1024, 1024, 16
P = 128
DK = D // H          # 64
NCH = D // P         # 8
QC = 512             # free-dim chunk (one PSUM bank)
NQC = S // QC        # 2
SCALE = 1.0 / float(np.sqrt(DK))  # 0.125
N_CORES = 8

BF16 = mybir.dt.bfloat16
F32 = mybir.dt.float32
ADD = mybir.AluOpType.add
MULT = mybir.AluOpType.mult
EXP = mybir.ActivationFunctionType.Exp

_CACHE = {}


def _build_nc():
    nc = bacc.Bacc(None)

    qT_d = nc.dram_tensor("qT", [NCH, P, S], BF16, kind="ExternalInput")
    keyT_d = nc.dram_tensor("keyT", [NCH, P, S], BF16, kind="ExternalInput")
    valT_d = nc.dram_tensor("valT", [NCH, P, S], BF16, kind="ExternalInput")
    wkT_d = nc.dram_tensor("wkT", [NCH, P, D], BF16, kind="ExternalInput")
    wvT_d = nc.dram_tensor("wvT", [NCH, P, D], BF16, kind="ExternalInput")
    woT_d = nc.dram_tensor("woT", [NCH, P, D], BF16, kind="ExternalInput")
    bk_d = nc.dram_tensor("bk", [D], F32, kind="ExternalInput")
    bv_d = nc.dram_tensor("bv", [D], F32, kind="ExternalInput")
    bo_d = nc.dram_tensor("bo", [D], F32, kind="ExternalInput")
    out_d = nc.dram_tensor("out", [S, D], F32, kind="ExternalOutput")

    with tile.TileContext(nc) as tc:
        with ExitStack() as ctx:
            const = ctx.enter_context(tc.tile_pool(name="const", bufs=1))
            big = ctx.enter_context(tc.tile_pool(name="big", bufs=1))
            wpool = ctx.enter_context(tc.tile_pool(name="wpool", bufs=1))
            ppool = ctx.enter_context(tc.tile_pool(name="ppool", bufs=2))
            opool = ctx.enter_context(tc.tile_pool(name="opool", bufs=4))
            rpool = ctx.enter_context(tc.tile_pool(name="rpool", bufs=3))
            proj_ps = ctx.enter_context(
                tc.tile_pool(name="proj_ps", bufs=4, space="PSUM")
            )
            sc_ps = ctx.enter_context(tc.tile_pool(name="sc_ps", bufs=2, space="PSUM"))
            xt_ps = ctx.enter_context(tc.tile_pool(name="xt_ps", bufs=2, space="PSUM"))

            # --- SBUF resident tensors ---
            qT = big.tile([P, NCH, S], BF16, tag="qT")
            keyT = big.tile([P, NCH, S], BF16, tag="share1")  # reused later by xT
            valT = big.tile([P, NCH, S], BF16, tag="valT")
            kT = big.tile([P, NCH, S], BF16, tag="kT")
            vpad = big.tile([P, NCH, H, DK + 1], BF16, tag="vpad")
            wk = wpool.tile([P, NCH, D], BF16, tag="wk")
            wv = wpool.tile([P, NCH, D], BF16, tag="wv")
            wo = wpool.tile([P, NCH, D], BF16, tag="wo")
            bk_sb = const.tile([P, NCH], F32, tag="bk")
            bv_b = const.tile([P, D], F32, tag="bv")
            bo_b = const.tile([P, D], F32, tag="bo")

            # --- input DMAs ---
            nc.sync.dma_start(out=qT[:], in_=qT_d[:].rearrange("c p f -> p c f"))
            nc.sync.dma_start(out=keyT[:], in_=keyT_d[:].rearrange("c p f -> p c f"))
            nc.sync.dma_start(out=valT[:], in_=valT_d[:].rearrange("c p f -> p c f"))
            nc.sync.dma_start(out=wk[:], in_=wkT_d[:].rearrange("c p f -> p c f"))
            nc.sync.dma_start(out=wv[:], in_=wvT_d[:].rearrange("c p f -> p c f"))
            nc.sync.dma_start(out=wo[:], in_=woT_d[:].rearrange("c p f -> p c f"))
            nc.sync.dma_start(out=bk_sb[:], in_=bk_d[:].rearrange("(c p) -> p c", p=P))
            nc.sync.dma_start(out=bv_b[:], in_=bv_d[:][None, :].to_broadcast((P, D)))
            nc.sync.dma_start(out=bo_b[:], in_=bo_d[:][None, :].to_broadcast((P, D)))

            # --- 1. K_T = Wk @ key.T + bk  (feature-major) ---
            for m in range(NCH):  # d_out tile
                ps = [proj_ps.tile([P, QC], F32, tag="pp", name=f"pp{j}") for j in range(NQC)]
                for c in range(NCH):  # d_in chunk (contraction)
                    st = wk[:, c, m * P : (m + 1) * P]
                    for j in range(NQC):
                        nc.tensor.matmul(
                            ps[j][:],
                            st,
                            keyT[:, c, j * QC : (j + 1) * QC],
                            start=(c == 0),
                            stop=(c == NCH - 1),
                        )
                for j in range(NQC):
                    nc.vector.tensor_scalar_add(
                        kT[:, m, j * QC : (j + 1) * QC], ps[j][:], bk_sb[:, m : m + 1]
                    )

            # --- 2. V = value @ Wv.T + bv (token-major, head-padded w/ ones) ---
            for t in range(NCH):  # s tile
                ps = [proj_ps.tile([P, QC], F32, tag="pp", name=f"pp{j}") for j in range(NQC)]
                for c in range(NCH):
                    st = valT[:, c, t * P : (t + 1) * P]
                    for j in range(NQC):
                        nc.tensor.matmul(
                            ps[j][:],
                            st,
                            wv[:, c, j * QC : (j + 1) * QC],
                            start=(c == 0),
                            stop=(c == NCH - 1),
                        )
                for h in range(H):
                    j, hl = divmod(h, QC // DK)  # which psum chunk, head within it
                    nc.vector.tensor_tensor(
                        vpad[:, t, h, 0:DK],
                        ps[j][:, hl * DK : (hl + 1) * DK],
                        bv_b[:, h * DK : (h + 1) * DK],
                        op=ADD,
                    )
                nc.vector.memset(vpad[:, t, :, DK : DK + 1], 1.0)

            # --- 3+4. per-head attention ---
            xT = big.tile([P, NCH, S], BF16, tag="share1")  # reuses keyT slot
            for h in range(H):
                ch, off = divmod(h, 2)
                off *= DK
                pt = ppool.tile([P, NCH, S], BF16, tag="pt")
                for kt in range(NCH):
                    st = kT[off : off + DK, ch, kt * P : (kt + 1) * P]
                    for j in range(NQC):
                        sp = sc_ps.tile([P, QC], F32, tag="sp")
                        nc.tensor.matmul(
                            sp[:],
                            st,
                            qT[off : off + DK, ch, j * QC : (j + 1) * QC],
                            start=True,
                            stop=True,
                        )
                        nc.scalar.activation(
                            pt[:, kt, j * QC : (j + 1) * QC], sp[:], EXP, scale=SCALE
                        )
                for j in range(NQC):
                    xp = xt_ps.tile([DK + 1, QC], F32, tag="xp")
                    for kc in range(NCH):
                        nc.tensor.matmul(
                            xp[:],
                            vpad[:, kc, h, :],
                            pt[:, kc, j * QC : (j + 1) * QC],
                            start=(kc == 0),
                            stop=(kc == NCH - 1),
                        )
                    rrow = rpool.tile([1, QC], F32, tag="rrow")
                    nc.vector.reciprocal(rrow[:], xp[DK : DK + 1, :])
                    rb = rpool.tile([DK, QC], F32, tag="rb")
                    nc.gpsimd.partition_broadcast(rb[:], rrow[:])
                    nc.vector.tensor_tensor(
                        xT[off : off + DK, ch, j * QC : (j + 1) * QC],
                        xp[0:DK, :],
                        rb[:],
                        op=MULT,
                    )

            # --- 5. O = x @ Wo.T + bo (token-major) ---
            for t in range(NCH):  # s tile
                ps = [proj_ps.tile([P, QC], F32, tag="pp", name=f"pp{j}") for j in range(NQC)]
                for c in range(NCH):
                    st = xT[:, c, t * P : (t + 1) * P]
                    for j in range(NQC):
                        nc.tensor.matmul(
                            ps[j][:],
                            st,
                            wo[:, c, j * QC : (j + 1) * QC],
                            start=(c == 0),
                            stop=(c == NCH - 1),
                        )
                for j in range(NQC):
                    ot = opool.tile([P, QC], F32, tag="ot")
                    nc.vector.tensor_tensor(
                        ot[:], ps[j][:], bo_b[:, j * QC : (j + 1) * QC], op=ADD
                    )
                    nc.sync.dma_start(
                        out=out_d[t * P : (t + 1) * P, j * QC : (j + 1) * QC],
                        in_=ot[:],
                    )

    nc.finalize()
    return nc


def get_nc():
    if "nc" not in _CACHE:
        _CACHE["nc"] = _build_nc()
    return _CACHE["nc"]


def _tp_bf16(a):
    """[X, Y] f32 -> transposed bf16 [NCH, P, Y]."""
    return (
        np.ascontiguousarray(np.asarray(a, dtype=np.float32).T)
        .astype(ml_dtypes.bfloat16)
        .reshape(NCH, P, -1)
    )


def make_in_maps(query, key, value, Wk, bk, Wv, bv, Wo, bo):
    wkT = _tp_bf16(Wk)
    wvT = _tp_bf16(Wv)
    woT = _tp_bf16(Wo)
    bk = np.asarray(bk, dtype=np.float32)
    bv = np.asarray(bv, dtype=np.float32)
    bo = np.asarray(bo, dtype=np.float32)
    in_maps = []
    for b in range(B):
        in_maps.append(
            {
                "qT": _tp_bf16(query[b]),
                "keyT": _tp_bf16(key[b]),
                "valT": _tp_bf16(value[b]),
                "wkT": wkT,
                "wvT": wvT,
                "woT": woT,
                "bk": bk,
                "bv": bv,
                "bo": bo,
            }
        )
    return in_maps


def run(trace=False, **inputs):
    from concourse.bass_utils import run_bass_kernel_spmd

    nc = get_nc()
    in_maps = make_in_maps(**inputs)
    res = run_bass_kernel_spmd(nc, in_maps, list(range(N_CORES)), trace=trace)
    out = np.stack([res.results[i]["out"] for i in range(N_CORES)], axis=0)
    return out, res


def kernel(**inputs):
    out, _ = run(trace=False, **inputs)
    return out


# revision 11
# speedup vs baseline: 1.4032x; 1.4032x over previous
"""Multi-headed attention (B=8, S=1024, D=1024, H=16) on 8 TRN2 NeuronCores.

Strategy: pure data parallelism over the batch — core b computes batch element b
end-to-end (no collectives). All matmuls in bf16 (fp32 PSUM accumulation).

Per-core dataflow (everything "T" is feature-major [D, S]):
  inputs (host-pretransposed, bf16): qT, keyT, valT, wkT, wvT, woT
  1. K_T[d_out, s]  = matmul(lhsT=wkT, rhs=keyT) + bk      (bias per-partition)
  2. V[s, d_out]    = matmul(lhsT=valT, rhs=wvT) + bv      -> packed [s, h, 65]
                      with a ones column per head (gives softmax denominators
                      for free inside the p@v matmul)
  3. per head h: scoresT[k, q] = matmul(lhsT=K_T_h[64,128], rhs=qT_h[64,512])
                 pT = exp(scoresT / 8)  (ACT; max-subtraction skipped — scores
                 are provably small for this problem)
  4. xT_h[65, q] accum = matmul(lhsT=[V_h | 1][128,65], rhs=pT[128,512]);
     row 64 = softmax denominator; normalize rows 0..63 by its reciprocal
     (partition-broadcast via DMA)
  5. O[s, d_out] = matmul(lhsT=xT, rhs=woT) + bo -> DMA out (f32)
"""

import numpy as np
import ml_dtypes

import concourse.bass as bass
import concourse.bacc as bacc
import concourse.mybir as mybir
import concourse.tile as tile
from contextlib import ExitStack

B, S, D, H = 8, 1024, 1024, 16
P = 128
DK = D // H          # 64
NCH = D // P         # 8
QC = 512             # free-dim chunk (one PSUM bank)
NQC = S // QC        # 2
SCALE = 1.0 / float(np.sqrt(DK))  # 0.125
N_CORES = 8

BF16 = mybir.dt.bfloat16
F32 = mybir.dt.float32
ADD = mybir.AluOpType.add
MULT = mybir.AluOpType.mult
EXP = mybir.ActivationFunctionType.Exp

_CACHE = {}


def _build_nc():
    nc = bacc.Bacc(None)

    qT_d = nc.dram_tensor("qT", [NCH, P, S], BF16, kind="ExternalInput")
    keyT_d = nc.dram_tensor("keyT", [NCH, P, S], BF16, kind="ExternalInput")
    valT_d = nc.dram_tensor("valT", [NCH, P, S], BF16, kind="ExternalInput")
    wkT_d = nc.dram_tensor("wkT", [NCH, P, D], BF16, kind="ExternalInput")
    wvT_d = nc.dram_tensor("wvT", [NCH, P, D], BF16, kind="ExternalInput")
    woT_d = nc.dram_tensor("woT", [NCH, P, D], BF16, kind="ExternalInput")
    bk_d = nc.dram_tensor("bk", [D], F32, kind="ExternalInput")
    bv_d = nc.dram_tensor("bv", [D], F32, kind="ExternalInput")
    bo_d = nc.dram_tensor("bo", [D], F32, kind="ExternalInput")
    out_d = nc.dram_tensor("out", [S, D], F32, kind="ExternalOutput")

    with tile.TileContext(nc) as tc:
        with ExitStack() as ctx:
            const = ctx.enter_context(tc.tile_pool(name="const", bufs=1))
            big = ctx.enter_context(tc.tile_pool(name="big", bufs=1))
            wpool = ctx.enter_context(tc.tile_pool(name="wpool", bufs=1))
            ppool = ctx.enter_context(tc.tile_pool(name="ppool", bufs=2))
            opool = ctx.enter_context(tc.tile_pool(name="opool", bufs=4))
            rpool = ctx.enter_context(tc.tile_pool(name="rpool", bufs=3))
            proj_ps = ctx.enter_context(
                tc.tile_pool(name="proj_ps", bufs=2, space="PSUM")
            )
            sc_ps = ctx.enter_context(tc.tile_pool(name="sc_ps", bufs=2, space="PSUM"))
            xt_ps = ctx.enter_context(tc.tile_pool(name="xt_ps", bufs=2, space="PSUM"))

            # --- SBUF resident tensors ---
            qT = big.tile([P, NCH, S], BF16, tag="qT")
            keyT = big.tile([P, NCH, S], BF16, tag="share1")  # reused later by xT
            valT = big.tile([P, NCH, S], BF16, tag="valT")
            kT = big.tile([P, NCH, S], BF16, tag="kT")
            vpad = big.tile([P, NCH, H, DK + 1], BF16, tag="vpad")
            wk = wpool.tile([P, NCH, D], BF16, tag="wk")
            wv = wpool.tile([P, NCH, D], BF16, tag="wv")
            wo = wpool.tile([P, NCH, D], BF16, tag="wo")
            bk_sb = const.tile([P, NCH], F32, tag="bk")
            bv_b = const.tile([P, D], F32, tag="bv")
            bo_b = const.tile([P, D], F32, tag="bo")

            # --- input DMAs (K-proj deps first so PE can start early) ---
            nc.sync.dma_start(out=keyT[:], in_=keyT_d[:].rearrange("c p f -> p c f"))
            nc.sync.dma_start(out=wk[:], in_=wkT_d[:].rearrange("c p f -> p c f"))
            nc.sync.dma_start(out=bk_sb[:], in_=bk_d[:].rearrange("(c p) -> p c", p=P))
            nc.sync.dma_start(out=valT[:], in_=valT_d[:].rearrange("c p f -> p c f"))
            nc.sync.dma_start(out=wv[:], in_=wvT_d[:].rearrange("c p f -> p c f"))
            nc.sync.dma_start(out=qT[:], in_=qT_d[:].rearrange("c p f -> p c f"))
            nc.sync.dma_start(out=wo[:], in_=woT_d[:].rearrange("c p f -> p c f"))
            nc.sync.dma_start(out=bv_b[:], in_=bv_d[:][None, :].to_broadcast((P, D)))
            nc.sync.dma_start(out=bo_b[:], in_=bo_d[:][None, :].to_broadcast((P, D)))

            # --- 1. K_T = Wk @ key.T + bk  (feature-major) ---
            for m in range(NCH):  # d_out tile
                ps = [proj_ps.tile([P, QC], F32, tag="pp", name=f"pp{j}") for j in range(NQC)]
                for c in range(NCH):  # d_in chunk (contraction)
                    st = wk[:, c, m * P : (m + 1) * P]
                    for j in range(NQC):
                        nc.tensor.matmul(
                            ps[j][:],
                            st,
                            keyT[:, c, j * QC : (j + 1) * QC],
                            start=(c == 0),
                            stop=(c == NCH - 1),
                        )
                for j in range(NQC):
                    nc.vector.tensor_scalar_add(
                        kT[:, m, j * QC : (j + 1) * QC], ps[j][:], bk_sb[:, m : m + 1]
                    )

            # --- 2. V = value @ Wv.T + bv (token-major, head-padded w/ ones) ---
            for t in range(NCH):  # s tile
                ps = [proj_ps.tile([P, QC], F32, tag="pp", name=f"pp{j}") for j in range(NQC)]
                for c in range(NCH):
                    st = valT[:, c, t * P : (t + 1) * P]
                    for j in range(NQC):
                        nc.tensor.matmul(
                            ps[j][:],
                            st,
                            wv[:, c, j * QC : (j + 1) * QC],
                            start=(c == 0),
                            stop=(c == NCH - 1),
                        )
                for h in range(H):
                    j, hl = divmod(h, QC // DK)  # which psum chunk, head within it
                    nc.vector.tensor_tensor(
                        vpad[:, t, h, 0:DK],
                        ps[j][:, hl * DK : (hl + 1) * DK],
                        bv_b[:, h * DK : (h + 1) * DK],
                        op=ADD,
                    )
                nc.vector.memset(vpad[:, t, :, DK : DK + 1], 1.0)

            # --- 3+4. per-head attention, q-chunk-outer, head-pipelined ---
            # PE is in-order: emit scores(h) before p@v(h-1) so the PE has
            # work while ACT chews through exp(h-1). O-projection s-tiles
            # for q-chunk j unlock once all heads finished chunk j; they are
            # interleaved into the following chunk (extra PE filler).
            xT = big.tile([P, NCH, S], BF16, tag="share1")  # reuses keyT slot

            def emit_scores(h, j):
                """scoresT + exp for head h, q-chunk j -> pt tile (returned)."""
                ch, off = divmod(h, 2)
                off *= DK
                pt = ppool.tile([P, NCH, QC], BF16, tag="pt", name=f"pt{h}_{j}")
                for kp in range(NCH // 2):  # kt pairs share one 2-bank psum
                    sp = sc_ps.tile([P, 2, QC], F32, tag="sp", name=f"sp{h}{j}{kp}")
                    for u in range(2):
                        kt = 2 * kp + u
                        nc.tensor.matmul(
                            sp[:, u, :],
                            kT[off : off + DK, ch, kt * P : (kt + 1) * P],
                            qT[off : off + DK, ch, j * QC : (j + 1) * QC],
                            start=True,
                            stop=True,
                        )
                    nc.scalar.activation(
                        pt[:, 2 * kp : 2 * kp + 2, :], sp[:], EXP, scale=SCALE
                    )
                return pt

            def emit_pv(h, j, pt):
                """p@v + softmax-normalize into xT for head h, q-chunk j."""
                ch, off = divmod(h, 2)
                off *= DK
                xp = xt_ps.tile([DK + 1, QC], F32, tag="xp", name=f"xp{h}_{j}")
                for kc in range(NCH):
                    nc.tensor.matmul(
                        xp[:],
                        vpad[:, kc, h, :],
                        pt[:, kc, :],
                        start=(kc == 0),
                        stop=(kc == NCH - 1),
                    )
                rrow = rpool.tile([1, QC], F32, tag="rrow", name=f"rr{h}_{j}")
                nc.vector.reciprocal(rrow[:], xp[DK : DK + 1, :])
                rb = rpool.tile([DK, QC], F32, tag="rb", name=f"rb{h}_{j}")
                nc.gpsimd.partition_broadcast(rb[:], rrow[:])
                nc.vector.tensor_tensor(
                    xT[off : off + DK, ch, j * QC : (j + 1) * QC],
                    xp[0:DK, :],
                    rb[:],
                    op=MULT,
                )

            def emit_oproj(t):
                """O = x @ Wo.T + bo for s-tile t (needs all of xT cols of t)."""
                ps = [
                    proj_ps.tile([P, QC], F32, tag="pp", name=f"op{t}_{j}")
                    for j in range(NQC)
                ]
                for c in range(NCH):
                    st = xT[:, c, t * P : (t + 1) * P]
                    for j in range(NQC):
                        nc.tensor.matmul(
                            ps[j][:],
                            st,
                            wo[:, c, j * QC : (j + 1) * QC],
                            start=(c == 0),
                            stop=(c == NCH - 1),
                        )
                for j in range(NQC):
                    ot = opool.tile([P, QC], F32, tag="ot", name=f"ot{t}_{j}")
                    nc.vector.tensor_tensor(
                        ot[:], ps[j][:], bo_b[:, j * QC : (j + 1) * QC], op=ADD
                    )
                    nc.sync.dma_start(
                        out=out_d[t * P : (t + 1) * P, j * QC : (j + 1) * QC],
                        in_=ot[:],
                    )

            for j in range(NQC):
                prev = None  # (h, pt) pending p@v
                for h in range(H):
                    pt = emit_scores(h, j)
                    if prev is not None:
                        emit_pv(prev[0], j, prev[1])
                    prev = (h, pt)
                    # during chunk j>0, interleave O-proj tiles of chunk j-1
                    if j > 0 and h in (3, 7, 11, 15):
                        emit_oproj((h - 3) // 4)
                emit_pv(prev[0], j, prev[1])
            # tail: O-proj s-tiles of the last q-chunk
            for t in range(4, NCH):
                emit_oproj(t)

    nc.finalize()
    return nc


def get_nc():
    if "nc" not in _CACHE:
        _CACHE["nc"] = _build_nc()
    return _CACHE["nc"]


def _tp_bf16(a):
    """[X, Y] f32 -> transposed bf16 [NCH, P, Y]."""
    return (
        np.ascontiguousarray(np.asarray(a, dtype=np.float32).T)
        .astype(ml_dtypes.bfloat16)
        .reshape(NCH, P, -1)
    )


def make_in_maps(query, key, value, Wk, bk, Wv, bv, Wo, bo):
    wkT = _tp_bf16(Wk)
    wvT = _tp_bf16(Wv)
    woT = _tp_bf16(Wo)
    bk = np.asarray(bk, dtype=np.float32)
    bv = np.asarray(bv, dtype=np.float32)
    bo = np.asarray(bo, dtype=np.float32)
    in_maps = []
    for b in range(B):
        in_maps.append(
            {
                "qT": _tp_bf16(query[b]),
                "keyT": _tp_bf16(key[b]),
                "valT": _tp_bf16(value[b]),
                "wkT": wkT,
                "wvT": wvT,
                "woT": woT,
                "bk": bk,
                "bv": bv,
                "bo": bo,
            }
        )
    return in_maps


def run(trace=False, **inputs):
    from concourse.bass_utils import run_bass_kernel_spmd

    nc = get_nc()
    in_maps = make_in_maps(**inputs)
    res = run_bass_kernel_spmd(nc, in_maps, list(range(N_CORES)), trace=trace)
    out = np.stack([res.results[i]["out"] for i in range(N_CORES)], axis=0)
    return out, res


def kernel(**inputs):
    out, _ = run(trace=False, **inputs)
    return out


# revision 20
# speedup vs baseline: 1.4164x; 1.0094x over previous
"""Multi-headed attention (B=8, S=1024, D=1024, H=16) on 8 TRN2 NeuronCores.

Strategy: pure data parallelism over the batch — core b computes batch element b
end-to-end (no collectives). All matmuls in bf16 (fp32 PSUM accumulation).

Per-core dataflow (everything "T" is feature-major [D, S]):
  inputs (host-pretransposed, bf16): qT, keyT, valT, wkT, wvT, woT
  1. K_T[d_out, s]  = matmul(lhsT=wkT, rhs=keyT) + bk      (bias per-partition)
  2. V[s, d_out]    = matmul(lhsT=valT, rhs=wvT) + bv      -> packed [s, h, 65]
                      with a ones column per head (gives softmax denominators
                      for free inside the p@v matmul)
  3. per head h: scoresT[k, q] = matmul(lhsT=K_T_h[64,128], rhs=qT_h[64,512])
                 pT = exp(scoresT / 8)  (ACT; max-subtraction skipped — scores
                 are provably small for this problem)
  4. xT_h[65, q] accum = matmul(lhsT=[V_h | 1][128,65], rhs=pT[128,512]);
     row 64 = softmax denominator; normalize rows 0..63 by its reciprocal
     (partition-broadcast via DMA)
  5. O[s, d_out] = matmul(lhsT=xT, rhs=woT) + bo -> DMA out (f32)
"""

import numpy as np
import ml_dtypes

import concourse.bass as bass
import concourse.bacc as bacc
import concourse.mybir as mybir
import concourse.tile as tile
from contextlib import ExitStack

B, S, D, H = 8, 1024, 1024, 16
P = 128
DK = D // H          # 64
NCH = D // P         # 8
QC = 512             # free-dim chunk (one PSUM bank)
NQC = S // QC        # 2
SCALE = 1.0 / float(np.sqrt(DK))  # 0.125
N_CORES = 8

BF16 = mybir.dt.bfloat16
F32 = mybir.dt.float32
ADD = mybir.AluOpType.add
MULT = mybir.AluOpType.mult
EXP = mybir.ActivationFunctionType.Exp

_CACHE = {}


def _build_nc():
    nc = bacc.Bacc(None)

    qT_d = nc.dram_tensor("qT", [NCH, P, S], BF16, kind="ExternalInput")
    keyT_d = nc.dram_tensor("keyT", [NCH, P, S], BF16, kind="ExternalInput")
    valT_d = nc.dram_tensor("valT", [NCH, P, S], BF16, kind="ExternalInput")
    wkT_d = nc.dram_tensor("wkT", [NCH, P, D], BF16, kind="ExternalInput")
    wvT_d = nc.dram_tensor("wvT", [NCH, P, D], BF16, kind="ExternalInput")
    woT_d = nc.dram_tensor("woT", [NCH, P, D], BF16, kind="ExternalInput")
    bk_d = nc.dram_tensor("bk", [D], F32, kind="ExternalInput")
    bv_d = nc.dram_tensor("bv", [D], F32, kind="ExternalInput")
    bo_d = nc.dram_tensor("bo", [D], F32, kind="ExternalInput")
    out_d = nc.dram_tensor("out", [S, D], F32, kind="ExternalOutput")

    with tile.TileContext(nc) as tc:
        with ExitStack() as ctx:
            const = ctx.enter_context(tc.tile_pool(name="const", bufs=1))
            big = ctx.enter_context(tc.tile_pool(name="big", bufs=1))
            wpool = ctx.enter_context(tc.tile_pool(name="wpool", bufs=1))
            ppool = ctx.enter_context(tc.tile_pool(name="ppool", bufs=2))
            opool = ctx.enter_context(tc.tile_pool(name="opool", bufs=4))
            rpool = ctx.enter_context(tc.tile_pool(name="rpool", bufs=2))
            xpool = ctx.enter_context(tc.tile_pool(name="xpool", bufs=8))
            proj_ps = ctx.enter_context(
                tc.tile_pool(name="proj_ps", bufs=2, space="PSUM")
            )
            sc_ps = ctx.enter_context(tc.tile_pool(name="sc_ps", bufs=2, space="PSUM"))
            xt_ps = ctx.enter_context(tc.tile_pool(name="xt_ps", bufs=2, space="PSUM"))

            # --- SBUF resident tensors ---
            qT = big.tile([P, NCH, S], BF16, tag="qT")
            keyT = big.tile([P, NCH, S], BF16, tag="share1")  # reused later by xT
            valT = big.tile([P, NCH, S], BF16, tag="valT")
            kT = big.tile([P, NCH, S], BF16, tag="kT")
            vpad = big.tile([P, NCH, H, DK + 1], BF16, tag="vpad")
            wk = wpool.tile([P, NCH, D], BF16, tag="wk")
            wv = wpool.tile([P, NCH, D], BF16, tag="wv")
            wo = wpool.tile([P, NCH, D], BF16, tag="wo")
            bk_sb = const.tile([P, NCH], F32, tag="bk")
            bv_b = const.tile([P, D], F32, tag="bv")
            bo_b = const.tile([P, D], F32, tag="bo")

            # --- input DMAs (K-proj deps first so PE can start early) ---
            nc.sync.dma_start(out=keyT[:], in_=keyT_d[:].rearrange("c p f -> p c f"))
            nc.sync.dma_start(out=wk[:], in_=wkT_d[:].rearrange("c p f -> p c f"))
            nc.sync.dma_start(out=bk_sb[:], in_=bk_d[:].rearrange("(c p) -> p c", p=P))
            nc.sync.dma_start(out=valT[:], in_=valT_d[:].rearrange("c p f -> p c f"))
            nc.sync.dma_start(out=wv[:], in_=wvT_d[:].rearrange("c p f -> p c f"))
            nc.sync.dma_start(out=qT[:], in_=qT_d[:].rearrange("c p f -> p c f"))
            nc.sync.dma_start(out=wo[:], in_=woT_d[:].rearrange("c p f -> p c f"))
            nc.sync.dma_start(out=bv_b[:], in_=bv_d[:][None, :].to_broadcast((P, D)))
            nc.sync.dma_start(out=bo_b[:], in_=bo_d[:][None, :].to_broadcast((P, D)))

            # --- 1. K_T = Wk @ key.T + bk  (feature-major) ---
            for m in range(NCH):  # d_out tile
                ps = [proj_ps.tile([P, QC], F32, tag="pp", name=f"pp{j}") for j in range(NQC)]
                for c in range(NCH):  # d_in chunk (contraction)
                    st = wk[:, c, m * P : (m + 1) * P]
                    for j in range(NQC):
                        nc.tensor.matmul(
                            ps[j][:],
                            st,
                            keyT[:, c, j * QC : (j + 1) * QC],
                            start=(c == 0),
                            stop=(c == NCH - 1),
                        )
                for j in range(NQC):
                    nc.vector.tensor_scalar_add(
                        kT[:, m, j * QC : (j + 1) * QC], ps[j][:], bk_sb[:, m : m + 1]
                    )

            # --- 2. V = value @ Wv.T + bv (token-major, head-padded w/ ones) ---
            for t in range(NCH):  # s tile
                ps = [proj_ps.tile([P, QC], F32, tag="pp", name=f"pp{j}") for j in range(NQC)]
                for c in range(NCH):
                    st = valT[:, c, t * P : (t + 1) * P]
                    for j in range(NQC):
                        nc.tensor.matmul(
                            ps[j][:],
                            st,
                            wv[:, c, j * QC : (j + 1) * QC],
                            start=(c == 0),
                            stop=(c == NCH - 1),
                        )
                for h in range(H):
                    j, hl = divmod(h, QC // DK)  # which psum chunk, head within it
                    nc.vector.tensor_tensor(
                        vpad[:, t, h, 0:DK],
                        ps[j][:, hl * DK : (hl + 1) * DK],
                        bv_b[:, h * DK : (h + 1) * DK],
                        op=ADD,
                    )
                nc.vector.memset(vpad[:, t, :, DK : DK + 1], 1.0)

            # --- 3+4. per-head attention, q-chunk-outer, head-pipelined ---
            # PE is in-order: emit scores(h) before p@v(h-1) so the PE has
            # work while ACT chews through exp(h-1). O-projection s-tiles
            # for q-chunk j unlock once all heads finished chunk j; they are
            # interleaved into the following chunk (extra PE filler).
            xT = big.tile([P, NCH, S], BF16, tag="share1")  # reuses keyT slot

            def emit_scores(h, j):
                """scoresT + exp for head h, q-chunk j -> pt tile (returned)."""
                ch, off = divmod(h, 2)
                off *= DK
                pt = ppool.tile([P, NCH, QC], BF16, tag="pt", name=f"pt{h}_{j}")
                for kp in range(NCH // 2):  # kt pairs share one 2-bank psum
                    sp = sc_ps.tile([P, 2, QC], F32, tag="sp", name=f"sp{h}{j}{kp}")
                    for u in range(2):
                        kt = 2 * kp + u
                        nc.tensor.matmul(
                            sp[:, u, :],
                            kT[off : off + DK, ch, kt * P : (kt + 1) * P],
                            qT[off : off + DK, ch, j * QC : (j + 1) * QC],
                            start=True,
                            stop=True,
                        )
                    nc.scalar.activation(
                        pt[:, 2 * kp : 2 * kp + 2, :], sp[:], EXP, scale=SCALE
                    )
                return pt

            def emit_pv(h, j, pt, dcols, xsb_map):
                """p@v for head h chunk j: unnormalized x -> SBUF, denom -> dcol.

                Engine APs need 32-aligned start partitions, so the 4 denoms
                of a batch land at partitions 0/32/64/96 of one collector."""
                xp = xt_ps.tile([DK + 1, QC], F32, tag="xp", name=f"xp{h}_{j}")
                for kc in range(NCH):
                    nc.tensor.matmul(
                        xp[:],
                        vpad[:, kc, h, :],
                        pt[:, kc, :],
                        start=(kc == 0),
                        stop=(kc == NCH - 1),
                    )
                xsb = xpool.tile([DK, QC], BF16, tag="xsb", name=f"xsb{h}_{j}")
                nc.vector.tensor_copy(xsb[:], xp[0:DK, :])
                b, r = divmod(h, 4)
                if r == 0:
                    dcols[b] = rpool.tile([97, QC], F32, tag="dcol", name=f"dc{j}_{b}")
                    nc.vector.memset(dcols[b][:], 1.0)  # only rows 0/32/64/96 matter
                nc.scalar.copy(dcols[b][32 * r : 32 * r + 1, :], xp[DK : DK + 1, :])
                xsb_map[h] = xsb

            def emit_norm_batch(j, hb, dcols, xsb_map):
                """reciprocal for heads hb..hb+3, broadcast + multiply into xT."""
                dcol = dcols[hb // 4]
                rcol = rpool.tile([97, QC], F32, tag="rcol", name=f"rc{j}_{hb}")
                nc.vector.reciprocal(rcol[:], dcol[:])
                for hh in range(hb, hb + 4):
                    ch, off = divmod(hh, 2)
                    off *= DK
                    r = 32 * (hh - hb)
                    # partition_broadcast ucode reads via Q7 core 0 -> input
                    # must sit at base partition 0; bounce the row there.
                    rb0 = rpool.tile([1, QC], F32, tag="rb0", name=f"rb0{j}_{hh}")
                    nc.vector.tensor_copy(rb0[:], rcol[r : r + 1, :])
                    rb = rpool.tile([DK, QC], F32, tag="rb", name=f"rb{j}_{hh}")
                    nc.gpsimd.partition_broadcast(rb[:], rb0[:])
                    nc.vector.tensor_tensor(
                        xT[off : off + DK, ch, j * QC : (j + 1) * QC],
                        xsb_map[hh][:],
                        rb[:],
                        op=MULT,
                    )

            def emit_oproj(t):
                """O = x @ Wo.T + bo for s-tile t (needs all of xT cols of t)."""
                ps = [
                    proj_ps.tile([P, QC], F32, tag="pp", name=f"op{t}_{j}")
                    for j in range(NQC)
                ]
                for c in range(NCH):
                    st = xT[:, c, t * P : (t + 1) * P]
                    for j in range(NQC):
                        nc.tensor.matmul(
                            ps[j][:],
                            st,
                            wo[:, c, j * QC : (j + 1) * QC],
                            start=(c == 0),
                            stop=(c == NCH - 1),
                        )
                for j in range(NQC):
                    ot = opool.tile([P, QC], F32, tag="ot", name=f"ot{t}_{j}")
                    nc.vector.tensor_tensor(
                        ot[:], ps[j][:], bo_b[:, j * QC : (j + 1) * QC], op=ADD
                    )
                    nc.sync.dma_start(
                        out=out_d[t * P : (t + 1) * P, j * QC : (j + 1) * QC],
                        in_=ot[:],
                    )

            for j in range(NQC):
                dcols = {}
                xsb_map = {}
                prev = None  # (h, pt) pending p@v
                for h in range(H):
                    pt = emit_scores(h, j)
                    if prev is not None:
                        emit_pv(prev[0], j, prev[1], dcols, xsb_map)
                    prev = (h, pt)
                    if h in (4, 8, 12):  # pv(h-1) done -> batch hb=h-4 ready
                        emit_norm_batch(j, h - 4, dcols, xsb_map)
                    # during chunk j>0, interleave O-proj tiles of chunk j-1
                    if j > 0 and h in (8, 10, 12, 14):
                        emit_oproj((h - 8) // 2)
                emit_pv(prev[0], j, prev[1], dcols, xsb_map)
                emit_norm_batch(j, 12, dcols, xsb_map)
            # tail: O-proj s-tiles of the last q-chunk
            for t in range(4, NCH):
                emit_oproj(t)

    nc.finalize()
    return nc


def get_nc():
    if "nc" not in _CACHE:
        _CACHE["nc"] = _build_nc()
    return _CACHE["nc"]


def _tp_bf16(a):
    """[X, Y] f32 -> transposed bf16 [NCH, P, Y]."""
    return (
        np.ascontiguousarray(np.asarray(a, dtype=np.float32).T)
        .astype(ml_dtypes.bfloat16)
        .reshape(NCH, P, -1)
    )


def make_in_maps(query, key, value, Wk, bk, Wv, bv, Wo, bo):
    wkT = _tp_bf16(Wk)
    wvT = _tp_bf16(Wv)
    woT = _tp_bf16(Wo)
    bk = np.asarray(bk, dtype=np.float32)
    bv = np.asarray(bv, dtype=np.float32)
    bo = np.asarray(bo, dtype=np.float32)
    in_maps = []
    for b in range(B):
        in_maps.append(
            {
                "qT": _tp_bf16(query[b]),
                "keyT": _tp_bf16(key[b]),
                "valT": _tp_bf16(value[b]),
                "wkT": wkT,
                "wvT": wvT,
                "woT": woT,
                "bk": bk,
                "bv": bv,
                "bo": bo,
            }
        )
    return in_maps


def run(trace=False, **inputs):
    from concourse.bass_utils import run_bass_kernel_spmd

    nc = get_nc()
    in_maps = make_in_maps(**inputs)
    res = run_bass_kernel_spmd(nc, in_maps, list(range(N_CORES)), trace=trace)
    out = np.stack([res.results[i]["out"] for i in range(N_CORES)], axis=0)
    return out, res


def kernel(**inputs):
    out, _ = run(trace=False, **inputs)
    return out


# revision 26
# speedup vs baseline: 1.5770x; 1.1134x over previous
"""Multi-headed attention (B=8, S=1024, D=1024, H=16) on 8 TRN2 NeuronCores.

Strategy: pure data parallelism over the batch — core b computes batch element b
end-to-end (no collectives). All matmuls in bf16 (fp32 PSUM accumulation).

Per-core dataflow (everything "T" is feature-major [D, S]):
  inputs (host-pretransposed, bf16): qT, keyT, valT, wkT, wvT, woT
  1. K_T[d_out, s]  = matmul(lhsT=wkT, rhs=keyT) + bk      (bias per-partition)
  2. V[s, d_out]    = matmul(lhsT=valT, rhs=wvT) + bv      -> packed [s, h, 65]
                      with a ones column per head (gives softmax denominators
                      for free inside the p@v matmul)
  3. per head h: scoresT[k, q] = matmul(lhsT=K_T_h[64,128], rhs=qT_h[64,512])
                 pT = exp(scoresT / 8)  (ACT; max-subtraction skipped — scores
                 are provably small for this problem)
  4. xT_h[65, q] accum = matmul(lhsT=[V_h | 1][128,65], rhs=pT[128,512]);
     row 64 = softmax denominator; normalize rows 0..63 by its reciprocal
     (partition-broadcast via DMA)
  5. O[s, d_out] = matmul(lhsT=xT, rhs=woT) + bo -> DMA out (f32)
"""

import numpy as np
import ml_dtypes

import concourse.bass as bass
import concourse.bacc as bacc
import concourse.mybir as mybir
import concourse.tile as tile
from contextlib import ExitStack

B, S, D, H = 8, 1024, 1024, 16
P = 128
DK = D // H          # 64
NCH = D // P         # 8
QC = 512             # free-dim chunk (one PSUM bank)
NQC = S // QC        # 2
SCALE = 1.0 / float(np.sqrt(DK))  # 0.125
N_CORES = 8

BF16 = mybir.dt.bfloat16
F32 = mybir.dt.float32
ADD = mybir.AluOpType.add
MULT = mybir.AluOpType.mult
EXP = mybir.ActivationFunctionType.Exp

_CACHE = {}


def _build_nc():
    nc = bacc.Bacc(None)

    qT_d = nc.dram_tensor("qT", [NCH, P, S], BF16, kind="ExternalInput")
    keyT_d = nc.dram_tensor("keyT", [NCH, P, S], BF16, kind="ExternalInput")
    valT_d = nc.dram_tensor("valT", [NCH, P, S], BF16, kind="ExternalInput")
    wkT_d = nc.dram_tensor("wkT", [NCH, P, D], BF16, kind="ExternalInput")
    wvT_d = nc.dram_tensor("wvT", [NCH, P, D], BF16, kind="ExternalInput")
    woT_d = nc.dram_tensor("woT", [NCH, P, D], BF16, kind="ExternalInput")
    bk_d = nc.dram_tensor("bk", [D], F32, kind="ExternalInput")
    bv_d = nc.dram_tensor("bv", [D], F32, kind="ExternalInput")
    bo_d = nc.dram_tensor("bo", [D], F32, kind="ExternalInput")
    out_d = nc.dram_tensor("out", [S, D], F32, kind="ExternalOutput")

    with tile.TileContext(nc) as tc:
        with ExitStack() as ctx:
            const = ctx.enter_context(tc.tile_pool(name="const", bufs=1))
            big = ctx.enter_context(tc.tile_pool(name="big", bufs=1))
            wpool = ctx.enter_context(tc.tile_pool(name="wpool", bufs=1))
            ppool = ctx.enter_context(tc.tile_pool(name="ppool", bufs=2))
            opool = ctx.enter_context(tc.tile_pool(name="opool", bufs=4))
            rpool = ctx.enter_context(tc.tile_pool(name="rpool", bufs=2))
            xpool = ctx.enter_context(tc.tile_pool(name="xpool", bufs=8))
            proj_ps = ctx.enter_context(
                tc.tile_pool(name="proj_ps", bufs=2, space="PSUM")
            )
            sc_ps = ctx.enter_context(tc.tile_pool(name="sc_ps", bufs=2, space="PSUM"))
            xt_ps = ctx.enter_context(tc.tile_pool(name="xt_ps", bufs=2, space="PSUM"))

            # --- SBUF resident tensors ---
            qT = big.tile([P, NCH, S], BF16, tag="qT")
            keyT = big.tile([P, NCH, S], BF16, tag="share1")  # reused later by xT
            valT = big.tile([P, NCH, S], BF16, tag="valT")
            kT = big.tile([P, NCH, S], BF16, tag="kT")
            vpad = big.tile([P, NCH, H, DK + 1], BF16, tag="vpad")
            wk = wpool.tile([P, NCH, D], BF16, tag="wk")
            wv = wpool.tile([P, NCH, D], BF16, tag="wv")
            wo = wpool.tile([P, NCH, D], BF16, tag="wo")
            bk_sb = const.tile([P, NCH], F32, tag="bk")
            bv_b = const.tile([P, D], F32, tag="bv")
            bo_b = const.tile([P, D], F32, tag="bo")

            # --- input DMAs (K-proj deps first so PE can start early) ---
            nc.sync.dma_start(out=keyT[:], in_=keyT_d[:].rearrange("c p f -> p c f"))
            nc.sync.dma_start(out=wk[:], in_=wkT_d[:].rearrange("c p f -> p c f"))
            nc.sync.dma_start(out=bk_sb[:], in_=bk_d[:].rearrange("(c p) -> p c", p=P))
            nc.sync.dma_start(out=valT[:], in_=valT_d[:].rearrange("c p f -> p c f"))
            nc.sync.dma_start(out=wv[:], in_=wvT_d[:].rearrange("c p f -> p c f"))
            nc.sync.dma_start(out=qT[:], in_=qT_d[:].rearrange("c p f -> p c f"))
            nc.sync.dma_start(out=wo[:], in_=woT_d[:].rearrange("c p f -> p c f"))
            nc.sync.dma_start(out=bv_b[:], in_=bv_d[:][None, :].to_broadcast((P, D)))
            nc.sync.dma_start(out=bo_b[:], in_=bo_d[:][None, :].to_broadcast((P, D)))

            # --- 1. K_T = Wk @ key.T + bk  (feature-major) ---
            for m in range(NCH):  # d_out tile
                ps = [proj_ps.tile([P, QC], F32, tag="pp", name=f"pp{j}") for j in range(NQC)]
                for c in range(NCH):  # d_in chunk (contraction)
                    st = wk[:, c, m * P : (m + 1) * P]
                    for j in range(NQC):
                        nc.tensor.matmul(
                            ps[j][:],
                            st,
                            keyT[:, c, j * QC : (j + 1) * QC],
                            start=(c == 0),
                            stop=(c == NCH - 1),
                        )
                for j in range(NQC):
                    nc.vector.tensor_scalar_add(
                        kT[:, m, j * QC : (j + 1) * QC], ps[j][:], bk_sb[:, m : m + 1]
                    )

            # --- 2. V = value @ Wv.T + bv (token-major, head-padded w/ ones) ---
            for t in range(NCH):  # s tile
                ps = [proj_ps.tile([P, QC], F32, tag="pp", name=f"pp{j}") for j in range(NQC)]
                for c in range(NCH):
                    st = valT[:, c, t * P : (t + 1) * P]
                    for j in range(NQC):
                        nc.tensor.matmul(
                            ps[j][:],
                            st,
                            wv[:, c, j * QC : (j + 1) * QC],
                            start=(c == 0),
                            stop=(c == NCH - 1),
                        )
                hpc = QC // DK  # heads per psum chunk
                for j in range(NQC):
                    nc.vector.tensor_tensor(
                        vpad[:, t, j * hpc : (j + 1) * hpc, 0:DK],
                        ps[j][:].rearrange("p (h d) -> p h d", d=DK),
                        bv_b[:, j * QC : (j + 1) * QC].rearrange(
                            "p (h d) -> p h d", d=DK
                        ),
                        op=ADD,
                    )
                nc.vector.memset(vpad[:, t, :, DK : DK + 1], 1.0)

            # --- 3+4. per-head attention, q-chunk-outer, head-pipelined ---
            # PE is in-order: emit scores(h) before p@v(h-1) so the PE has
            # work while ACT chews through exp(h-1). O-projection s-tiles
            # for q-chunk j unlock once all heads finished chunk j; they are
            # interleaved into the following chunk (extra PE filler).
            xT = big.tile([P, NCH, S], BF16, tag="share1")  # reuses keyT slot

            def emit_scores(h, j):
                """scoresT + exp for head h, q-chunk j -> pt tile (returned)."""
                ch, off = divmod(h, 2)
                off *= DK
                pt = ppool.tile([P, NCH, QC], BF16, tag="pt", name=f"pt{h}_{j}")
                for kp in range(NCH // 2):  # kt pairs share one 2-bank psum
                    sp = sc_ps.tile([P, 2, QC], F32, tag="sp", name=f"sp{h}{j}{kp}")
                    for u in range(2):
                        kt = 2 * kp + u
                        nc.tensor.matmul(
                            sp[:, u, :],
                            kT[off : off + DK, ch, kt * P : (kt + 1) * P],
                            qT[off : off + DK, ch, j * QC : (j + 1) * QC],
                            start=True,
                            stop=True,
                        )
                    nc.scalar.activation(
                        pt[:, 2 * kp : 2 * kp + 2, :], sp[:], EXP, scale=SCALE
                    )
                return pt

            def emit_pv(h, j, pt, dcols, xsb_map):
                """p@v for head h chunk j: unnormalized x -> SBUF, denom -> dcol.

                Engine APs need 32-aligned start partitions, so the 4 denoms
                of a batch land at partitions 0/32/64/96 of one collector."""
                xp = xt_ps.tile([DK + 1, QC], F32, tag="xp", name=f"xp{h}_{j}")
                for kc in range(NCH):
                    nc.tensor.matmul(
                        xp[:],
                        vpad[:, kc, h, :],
                        pt[:, kc, :],
                        start=(kc == 0),
                        stop=(kc == NCH - 1),
                    )
                xsb = xpool.tile([DK, QC], BF16, tag="xsb", name=f"xsb{h}_{j}")
                nc.vector.tensor_copy(xsb[:], xp[0:DK, :])
                b, r = divmod(h, 4)
                if r == 0:
                    dcols[b] = rpool.tile([97, QC], F32, tag="dcol", name=f"dc{j}_{b}")
                    nc.vector.memset(dcols[b][:], 1.0)  # only rows 0/32/64/96 matter
                nc.vector.tensor_copy(dcols[b][32 * r : 32 * r + 1, :], xp[DK : DK + 1, :])
                xsb_map[h] = xsb

            def emit_norm_recip(j, hb, dcols, rb_map):
                """reciprocal for heads hb..hb+3 + partition-broadcasts."""
                dcol = dcols[hb // 4]
                rcol = rpool.tile([97, QC], F32, tag="rcol", name=f"rc{j}_{hb}")
                nc.vector.reciprocal(rcol[:], dcol[:])
                for hh in range(hb, hb + 4):
                    r = 32 * (hh - hb)
                    # partition_broadcast ucode reads via Q7 core 0 -> input
                    # must sit at base partition 0; bounce the row there.
                    rb0 = rpool.tile([1, QC], F32, tag="rb0", name=f"rb0{j}_{hh}")
                    nc.vector.tensor_copy(rb0[:], rcol[r : r + 1, :])
                    rb = rpool.tile([DK, QC], F32, tag="rb", name=f"rb{j}_{hh}", bufs=4)
                    nc.gpsimd.partition_broadcast(rb[:], rb0[:])
                    rb_map[hh] = rb

            def emit_norm_mults(j, hb, xsb_map, rb_map):
                """deferred multiplies (by now the broadcasts are long done)."""
                for hh in range(hb, hb + 4):
                    ch, off = divmod(hh, 2)
                    off *= DK
                    nc.vector.tensor_tensor(
                        xT[off : off + DK, ch, j * QC : (j + 1) * QC],
                        xsb_map[hh][:],
                        rb_map[hh][:],
                        op=MULT,
                    )

            def emit_oproj(t):
                """O = x @ Wo.T + bo for s-tile t (needs all of xT cols of t)."""
                ps = [
                    proj_ps.tile([P, QC], F32, tag="pp", name=f"op{t}_{j}")
                    for j in range(NQC)
                ]
                for c in range(NCH):
                    st = xT[:, c, t * P : (t + 1) * P]
                    for j in range(NQC):
                        nc.tensor.matmul(
                            ps[j][:],
                            st,
                            wo[:, c, j * QC : (j + 1) * QC],
                            start=(c == 0),
                            stop=(c == NCH - 1),
                        )
                for j in range(NQC):
                    ot = opool.tile([P, QC], F32, tag="ot", name=f"ot{t}_{j}")
                    nc.vector.tensor_tensor(
                        ot[:], ps[j][:], bo_b[:, j * QC : (j + 1) * QC], op=ADD
                    )
                    nc.sync.dma_start(
                        out=out_d[t * P : (t + 1) * P, j * QC : (j + 1) * QC],
                        in_=ot[:],
                    )

            for j in range(NQC):
                dcols = {}
                xsb_map = {}
                rb_map = {}
                prev = None  # (h, pt) pending p@v
                for h in range(H):
                    pt = emit_scores(h, j)
                    if prev is not None:
                        emit_pv(prev[0], j, prev[1], dcols, xsb_map)
                    prev = (h, pt)
                    if h in (4, 8, 12):  # pv(h-1) done -> batch hb=h-4 ready
                        emit_norm_recip(j, h - 4, dcols, rb_map)
                    if h in (6, 10, 14):  # broadcasts of batch (h-6)/4 done
                        emit_norm_mults(j, h - 6, xsb_map, rb_map)
                    # during chunk j>0, interleave O-proj tiles of chunk j-1
                    if j > 0 and h in (8, 10, 12, 14):
                        emit_oproj((h - 8) // 2)
                emit_pv(prev[0], j, prev[1], dcols, xsb_map)
                emit_norm_recip(j, 12, dcols, rb_map)
                emit_norm_mults(j, 12, xsb_map, rb_map)
            # tail: O-proj s-tiles of the last q-chunk
            for t in range(4, NCH):
                emit_oproj(t)

    nc.finalize()
    return nc


def get_nc():
    if "nc" not in _CACHE:
        _CACHE["nc"] = _build_nc()
    return _CACHE["nc"]


def _tp_bf16(a):
    """[X, Y] f32 -> transposed bf16 [NCH, P, Y]."""
    return (
        np.ascontiguousarray(np.asarray(a, dtype=np.float32).T)
        .astype(ml_dtypes.bfloat16)
        .reshape(NCH, P, -1)
    )


def make_in_maps(query, key, value, Wk, bk, Wv, bv, Wo, bo):
    wkT = _tp_bf16(Wk)
    wvT = _tp_bf16(Wv)
    woT = _tp_bf16(Wo)
    bk = np.asarray(bk, dtype=np.float32)
    bv = np.asarray(bv, dtype=np.float32)
    bo = np.asarray(bo, dtype=np.float32)
    in_maps = []
    for b in range(B):
        in_maps.append(
            {
                "qT": _tp_bf16(query[b]),
                "keyT": _tp_bf16(key[b]),
                "valT": _tp_bf16(value[b]),
                "wkT": wkT,
                "wvT": wvT,
                "woT": woT,
                "bk": bk,
                "bv": bv,
                "bo": bo,
            }
        )
    return in_maps


def run(trace=False, **inputs):
    from concourse.bass_utils import run_bass_kernel_spmd

    nc = get_nc()
    in_maps = make_in_maps(**inputs)
    res = run_bass_kernel_spmd(nc, in_maps, list(range(N_CORES)), trace=trace)
    out = np.stack([res.results[i]["out"] for i in range(N_CORES)], axis=0)
    return out, res


def kernel(**inputs):
    out, _ = run(trace=False, **inputs)
    return out


# revision 31
# speedup vs baseline: 1.5861x; 1.0058x over previous
"""Multi-headed attention (B=8, S=1024, D=1024, H=16) on 8 TRN2 NeuronCores.

Strategy: pure data parallelism over the batch — core b computes batch element b
end-to-end (no collectives). All matmuls in bf16 (fp32 PSUM accumulation).

Per-core dataflow (everything "T" is feature-major [D, S]):
  inputs (host-pretransposed, bf16): qT, keyT, valT, wkT, wvT, woT
  1. K_T[d_out, s]  = matmul(lhsT=wkT, rhs=keyT) + bk      (bias per-partition)
  2. V[s, d_out]    = matmul(lhsT=valT, rhs=wvT) + bv      -> packed [s, h, 65]
                      with a ones column per head (gives softmax denominators
                      for free inside the p@v matmul)
  3. per head h: scoresT[k, q] = matmul(lhsT=K_T_h[64,128], rhs=qT_h[64,512])
                 pT = exp(scoresT / 8)  (ACT; max-subtraction skipped — scores
                 are provably small for this problem)
  4. xT_h[65, q] accum = matmul(lhsT=[V_h | 1][128,65], rhs=pT[128,512]);
     row 64 = softmax denominator; normalize rows 0..63 by its reciprocal
     (partition-broadcast via DMA)
  5. O[s, d_out] = matmul(lhsT=xT, rhs=woT) + bo -> DMA out (f32)
"""

import numpy as np
import ml_dtypes

import concourse.bass as bass
import concourse.bacc as bacc
import concourse.mybir as mybir
import concourse.tile as tile
from contextlib import ExitStack

B, S, D, H = 8, 1024, 1024, 16
P = 128
DK = D // H          # 64
NCH = D // P         # 8
QC = 512             # free-dim chunk (one PSUM bank)
NQC = S // QC        # 2
SCALE = 1.0 / float(np.sqrt(DK))  # 0.125
N_CORES = 8

BF16 = mybir.dt.bfloat16
F32 = mybir.dt.float32
ADD = mybir.AluOpType.add
MULT = mybir.AluOpType.mult
EXP = mybir.ActivationFunctionType.Exp

_CACHE = {}


def _build_nc():
    nc = bacc.Bacc(None)

    qT_d = nc.dram_tensor("qT", [NCH, P, S], BF16, kind="ExternalInput")
    keyT_d = nc.dram_tensor("keyT", [NCH, P, S], BF16, kind="ExternalInput")
    valT_d = nc.dram_tensor("valT", [NCH, P, S], BF16, kind="ExternalInput")
    wkT_d = nc.dram_tensor("wkT", [NCH, P, D], BF16, kind="ExternalInput")
    wvT_d = nc.dram_tensor("wvT", [NCH, P, D], BF16, kind="ExternalInput")
    woT_d = nc.dram_tensor("woT", [NCH, P, D], BF16, kind="ExternalInput")
    bk_d = nc.dram_tensor("bk", [D], F32, kind="ExternalInput")
    bv_d = nc.dram_tensor("bv", [D], F32, kind="ExternalInput")
    bo_d = nc.dram_tensor("bo", [D], F32, kind="ExternalInput")
    out_d = nc.dram_tensor("out", [S, D], F32, kind="ExternalOutput")

    with tile.TileContext(nc) as tc:
        with ExitStack() as ctx:
            const = ctx.enter_context(tc.tile_pool(name="const", bufs=1))
            big = ctx.enter_context(tc.tile_pool(name="big", bufs=1))
            wpool = ctx.enter_context(tc.tile_pool(name="wpool", bufs=1))
            ppool = ctx.enter_context(tc.tile_pool(name="ppool", bufs=2))
            opool = ctx.enter_context(tc.tile_pool(name="opool", bufs=4))
            rpool = ctx.enter_context(tc.tile_pool(name="rpool", bufs=2))
            xpool = ctx.enter_context(tc.tile_pool(name="xpool", bufs=8))
            proj_ps = ctx.enter_context(
                tc.tile_pool(name="proj_ps", bufs=2, space="PSUM")
            )
            sc_ps = ctx.enter_context(tc.tile_pool(name="sc_ps", bufs=2, space="PSUM"))
            xt_ps = ctx.enter_context(tc.tile_pool(name="xt_ps", bufs=2, space="PSUM"))

            # --- SBUF resident tensors ---
            qT = big.tile([P, NCH, S], BF16, tag="qT")
            keyT = big.tile([P, NCH, S], BF16, tag="share1")  # reused later by xT
            valT = big.tile([P, NCH, S], BF16, tag="valT")
            kT = big.tile([P, NCH, S], BF16, tag="kT")
            vpad = big.tile([P, NCH, H, DK + 1], BF16, tag="vpad")
            wk = wpool.tile([P, NCH, D], BF16, tag="wk")
            wv = wpool.tile([P, NCH, D], BF16, tag="wv")
            wo = wpool.tile([P, NCH, D], BF16, tag="wo")
            bk_sb = const.tile([P, NCH], F32, tag="bk")
            bv_b = const.tile([P, D], F32, tag="bv")
            bo_b = const.tile([P, D], F32, tag="bo")

            # --- input DMAs (V-proj deps first: it must fully precede p@v) ---
            nc.sync.dma_start(out=valT[:], in_=valT_d[:].rearrange("c p f -> p c f"))
            nc.sync.dma_start(out=wv[:], in_=wvT_d[:].rearrange("c p f -> p c f"))
            nc.sync.dma_start(out=bv_b[:], in_=bv_d[:][None, :].to_broadcast((P, D)))
            nc.sync.dma_start(out=keyT[:], in_=keyT_d[:].rearrange("c p f -> p c f"))
            nc.sync.dma_start(out=wk[:], in_=wkT_d[:].rearrange("c p f -> p c f"))
            nc.sync.dma_start(out=bk_sb[:], in_=bk_d[:].rearrange("(c p) -> p c", p=P))
            nc.sync.dma_start(out=qT[:], in_=qT_d[:].rearrange("c p f -> p c f"))
            nc.sync.dma_start(out=wo[:], in_=woT_d[:].rearrange("c p f -> p c f"))
            nc.sync.dma_start(out=bo_b[:], in_=bo_d[:][None, :].to_broadcast((P, D)))

            # --- 1. K_T = Wk @ key.T + bk  (feature-major) ---
            # m-tile 0 runs in the prologue; tiles 1..7 are interleaved into
            # attention chunk 0 as PE filler (head pair m needs only tile m).
            def emit_ktproj(m):
                ps = [
                    proj_ps.tile([P, QC], F32, tag="pp", name=f"kp{m}_{j}")
                    for j in range(NQC)
                ]
                for c in range(NCH):  # d_in chunk (contraction)
                    st = wk[:, c, m * P : (m + 1) * P]
                    for j in range(NQC):
                        nc.tensor.matmul(
                            ps[j][:],
                            st,
                            keyT[:, c, j * QC : (j + 1) * QC],
                            start=(c == 0),
                            stop=(c == NCH - 1),
                        )
                for j in range(NQC):
                    nc.vector.tensor_scalar_add(
                        kT[:, m, j * QC : (j + 1) * QC], ps[j][:], bk_sb[:, m : m + 1]
                    )

            # --- 2. V = value @ Wv.T + bv (token-major, head-padded w/ ones) ---
            for t in range(NCH):  # s tile
                ps = [proj_ps.tile([P, QC], F32, tag="pp", name=f"pp{j}") for j in range(NQC)]
                for c in range(NCH):
                    st = valT[:, c, t * P : (t + 1) * P]
                    for j in range(NQC):
                        nc.tensor.matmul(
                            ps[j][:],
                            st,
                            wv[:, c, j * QC : (j + 1) * QC],
                            start=(c == 0),
                            stop=(c == NCH - 1),
                        )
                hpc = QC // DK  # heads per psum chunk
                for j in range(NQC):
                    nc.vector.tensor_tensor(
                        vpad[:, t, j * hpc : (j + 1) * hpc, 0:DK],
                        ps[j][:].rearrange("p (h d) -> p h d", d=DK),
                        bv_b[:, j * QC : (j + 1) * QC].rearrange(
                            "p (h d) -> p h d", d=DK
                        ),
                        op=ADD,
                    )
                nc.vector.memset(vpad[:, t, :, DK : DK + 1], 1.0)

            emit_ktproj(0)  # head pair 0's K_T; tiles 1..7 interleave below

            # --- 3+4. per-head attention, q-chunk-outer, head-pipelined ---
            # PE is in-order: emit scores(h) before p@v(h-1) so the PE has
            # work while ACT chews through exp(h-1). O-projection s-tiles
            # for q-chunk j unlock once all heads finished chunk j; they are
            # interleaved into the following chunk (extra PE filler).
            xT = big.tile([P, NCH, S], BF16, tag="share1")  # reuses keyT slot

            def emit_scores(h, j):
                """scoresT + exp for head h, q-chunk j -> pt tile (returned)."""
                ch, off = divmod(h, 2)
                off *= DK
                pt = ppool.tile([P, NCH, QC], BF16, tag="pt", name=f"pt{h}_{j}")
                for kp in range(NCH // 2):  # kt pairs share one 2-bank psum
                    sp = sc_ps.tile([P, 2, QC], F32, tag="sp", name=f"sp{h}{j}{kp}")
                    for u in range(2):
                        kt = 2 * kp + u
                        nc.tensor.matmul(
                            sp[:, u, :],
                            kT[off : off + DK, ch, kt * P : (kt + 1) * P],
                            qT[off : off + DK, ch, j * QC : (j + 1) * QC],
                            start=True,
                            stop=True,
                        )
                    nc.scalar.activation(
                        pt[:, 2 * kp : 2 * kp + 2, :], sp[:], EXP, scale=SCALE
                    )
                return pt

            def emit_pv(h, j, pt, dcols, xsb_map):
                """p@v for head h chunk j: unnormalized x -> SBUF, denom -> dcol.

                Engine APs need 32-aligned start partitions, so the 4 denoms
                of a batch land at partitions 0/32/64/96 of one collector."""
                xp = xt_ps.tile([DK + 1, QC], F32, tag="xp", name=f"xp{h}_{j}")
                for kc in range(NCH):
                    nc.tensor.matmul(
                        xp[:],
                        vpad[:, kc, h, :],
                        pt[:, kc, :],
                        start=(kc == 0),
                        stop=(kc == NCH - 1),
                    )
                xsb = xpool.tile([DK, QC], BF16, tag="xsb", name=f"xsb{h}_{j}")
                nc.vector.tensor_copy(xsb[:], xp[0:DK, :])
                b, r = divmod(h, 4)
                if r == 0:
                    dcols[b] = rpool.tile([97, QC], F32, tag="dcol", name=f"dc{j}_{b}")
                    nc.vector.memset(dcols[b][:], 1.0)  # only rows 0/32/64/96 matter
                nc.vector.tensor_copy(dcols[b][32 * r : 32 * r + 1, :], xp[DK : DK + 1, :])
                xsb_map[h] = xsb

            def emit_recip_half(j, hb, half, dcols, rcols):
                """half of the batch reciprocal (split so the DVE queue never
                blocks >~1.7us in front of the xp-slot-freeing copies)."""
                if half == 0:
                    rcols[hb // 4] = rpool.tile(
                        [97, QC], F32, tag="rcol", name=f"rc{j}_{hb}"
                    )
                sl = slice(half * (QC // 2), (half + 1) * (QC // 2))
                nc.vector.reciprocal(rcols[hb // 4][:, sl], dcols[hb // 4][:, sl])

            def emit_bcasts(j, hb, rcols, rb_map):
                for hh in range(hb, hb + 4):
                    r = 32 * (hh - hb)
                    # partition_broadcast ucode reads via Q7 core 0 -> input
                    # must sit at base partition 0; bounce the row there (ACT).
                    rb0 = rpool.tile([1, QC], F32, tag="rb0", name=f"rb0{j}_{hh}")
                    nc.scalar.copy(rb0[:], rcols[hb // 4][r : r + 1, :])
                    rb = rpool.tile([DK, QC], F32, tag="rb", name=f"rb{j}_{hh}", bufs=4)
                    nc.gpsimd.partition_broadcast(rb[:], rb0[:])
                    rb_map[hh] = rb

            def emit_norm_mults(j, hb, xsb_map, rb_map):
                """deferred multiplies (by now the broadcasts are long done)."""
                for hh in range(hb, hb + 4):
                    ch, off = divmod(hh, 2)
                    off *= DK
                    nc.vector.tensor_tensor(
                        xT[off : off + DK, ch, j * QC : (j + 1) * QC],
                        xsb_map[hh][:],
                        rb_map[hh][:],
                        op=MULT,
                    )

            def emit_oproj(t):
                """O = x @ Wo.T + bo for s-tile t (needs all of xT cols of t)."""
                ps = [
                    proj_ps.tile([P, QC], F32, tag="pp", name=f"op{t}_{j}")
                    for j in range(NQC)
                ]
                for c in range(NCH):
                    st = xT[:, c, t * P : (t + 1) * P]
                    for j in range(NQC):
                        nc.tensor.matmul(
                            ps[j][:],
                            st,
                            wo[:, c, j * QC : (j + 1) * QC],
                            start=(c == 0),
                            stop=(c == NCH - 1),
                        )
                for j in range(NQC):
                    ot = opool.tile([P, QC], F32, tag="ot", name=f"ot{t}_{j}")
                    nc.vector.tensor_tensor(
                        ot[:], ps[j][:], bo_b[:, j * QC : (j + 1) * QC], op=ADD
                    )
                    nc.sync.dma_start(
                        out=out_d[t * P : (t + 1) * P, j * QC : (j + 1) * QC],
                        in_=ot[:],
                    )

            for j in range(NQC):
                dcols = {}
                xsb_map = {}
                rcols = {}
                rb_map = {}
                prev = None  # (h, pt) pending p@v
                for h in range(H):
                    # PE filler: chunk 0 takes K-proj tiles, chunk 1 O-proj
                    if j == 0 and h % 2 == 1 and h < 15:
                        emit_ktproj((h + 1) // 2)
                    if j > 0 and h in (4, 6, 8, 10):
                        emit_oproj((h - 4) // 2)
                    pt = emit_scores(h, j)
                    if prev is not None:
                        emit_pv(prev[0], j, prev[1], dcols, xsb_map)
                    prev = (h, pt)
                    if h in (4, 8, 12):  # pv(h-1) done -> batch hb=h-4 ready
                        emit_recip_half(j, h - 4, 0, dcols, rcols)
                    if h in (5, 9, 13):
                        emit_recip_half(j, h - 5, 1, dcols, rcols)
                        emit_bcasts(j, h - 5, rcols, rb_map)
                    if h in (7, 11, 15):  # broadcasts of batch (h-7)/4 done
                        emit_norm_mults(j, h - 7, xsb_map, rb_map)
                emit_pv(prev[0], j, prev[1], dcols, xsb_map)
                for half in range(2):
                    emit_recip_half(j, 12, half, dcols, rcols)
                emit_bcasts(j, 12, rcols, rb_map)
                emit_norm_mults(j, 12, xsb_map, rb_map)
            # tail: O-proj s-tiles of the last q-chunk
            for t in range(4, NCH):
                emit_oproj(t)

    nc.finalize()
    return nc


def get_nc():
    if "nc" not in _CACHE:
        _CACHE["nc"] = _build_nc()
    return _CACHE["nc"]


def _tp_bf16(a):
    """[X, Y] f32 -> transposed bf16 [NCH, P, Y]."""
    return (
        np.ascontiguousarray(np.asarray(a, dtype=np.float32).T)
        .astype(ml_dtypes.bfloat16)
        .reshape(NCH, P, -1)
    )


def make_in_maps(query, key, value, Wk, bk, Wv, bv, Wo, bo):
    wkT = _tp_bf16(Wk)
    wvT = _tp_bf16(Wv)
    woT = _tp_bf16(Wo)
    bk = np.asarray(bk, dtype=np.float32)
    bv = np.asarray(bv, dtype=np.float32)
    bo = np.asarray(bo, dtype=np.float32)
    in_maps = []
    for b in range(B):
        in_maps.append(
            {
                "qT": _tp_bf16(query[b]),
                "keyT": _tp_bf16(key[b]),
                "valT": _tp_bf16(value[b]),
                "wkT": wkT,
                "wvT": wvT,
                "woT": woT,
                "bk": bk,
                "bv": bv,
                "bo": bo,
            }
        )
    return in_maps


def run(trace=False, **inputs):
    from concourse.bass_utils import run_bass_kernel_spmd

    nc = get_nc()
    in_maps = make_in_maps(**inputs)
    res = run_bass_kernel_spmd(nc, in_maps, list(range(N_CORES)), trace=trace)
    out = np.stack([res.results[i]["out"] for i in range(N_CORES)], axis=0)
    return out, res


def kernel(**inputs):
    out, _ = run(trace=False, **inputs)
    return out


# revision 34
# speedup vs baseline: 1.6289x; 1.0270x over previous
"""Multi-headed attention (B=8, S=1024, D=1024, H=16) on 8 TRN2 NeuronCores.

Strategy: pure data parallelism over the batch — core b computes batch element b
end-to-end (no collectives). All matmuls in bf16 (fp32 PSUM accumulation).

Per-core dataflow (everything "T" is feature-major [D, S]):
  inputs (host-pretransposed, bf16): qT, keyT, valT, wkT, wvT, woT
  1. K_T[d_out, s]  = matmul(lhsT=wkT, rhs=keyT) + bk      (bias per-partition)
  2. V[s, d_out]    = matmul(lhsT=valT, rhs=wvT) + bv      -> packed [s, h, 65]
                      with a ones column per head (gives softmax denominators
                      for free inside the p@v matmul)
  3. per head h: scoresT[k, q] = matmul(lhsT=K_T_h[64,128], rhs=qT_h[64,512])
                 pT = exp(scoresT / 8)  (ACT; max-subtraction skipped — scores
                 are provably small for this problem)
  4. xT_h[65, q] accum = matmul(lhsT=[V_h | 1][128,65], rhs=pT[128,512]);
     row 64 = softmax denominator; normalize rows 0..63 by its reciprocal
     (partition-broadcast via DMA)
  5. O[s, d_out] = matmul(lhsT=xT, rhs=woT) + bo -> DMA out (f32)
"""

import numpy as np
import ml_dtypes

import concourse.bass as bass
import concourse.bacc as bacc
import concourse.mybir as mybir
import concourse.tile as tile
from contextlib import ExitStack

B, S, D, H = 8, 1024, 1024, 16
P = 128
DK = D // H          # 64
NCH = D // P         # 8
QC = 512             # free-dim chunk (one PSUM bank)
NQC = S // QC        # 2
SCALE = 1.0 / float(np.sqrt(DK))  # 0.125
N_CORES = 8

BF16 = mybir.dt.bfloat16
F32 = mybir.dt.float32
ADD = mybir.AluOpType.add
MULT = mybir.AluOpType.mult
EXP = mybir.ActivationFunctionType.Exp

_CACHE = {}


def _build_nc():
    nc = bacc.Bacc(None)

    qT_d = nc.dram_tensor("qT", [NCH, P, S], BF16, kind="ExternalInput")
    keyT_d = nc.dram_tensor("keyT", [NCH, P, S], BF16, kind="ExternalInput")
    valT_d = nc.dram_tensor("valT", [NCH, P, S], BF16, kind="ExternalInput")
    wkT_d = nc.dram_tensor("wkT", [NCH, P, D], BF16, kind="ExternalInput")
    wvT_d = nc.dram_tensor("wvT", [NCH, P, D], BF16, kind="ExternalInput")
    woT_d = nc.dram_tensor("woT", [NCH, P, D], BF16, kind="ExternalInput")
    bk_d = nc.dram_tensor("bk", [D], F32, kind="ExternalInput")
    bv_d = nc.dram_tensor("bv", [D], F32, kind="ExternalInput")
    bo_d = nc.dram_tensor("bo", [D], F32, kind="ExternalInput")
    out_d = nc.dram_tensor("out", [S, D], F32, kind="ExternalOutput")

    with tile.TileContext(nc) as tc:
        with ExitStack() as ctx:
            const = ctx.enter_context(tc.tile_pool(name="const", bufs=1))
            big = ctx.enter_context(tc.tile_pool(name="big", bufs=1))
            wpool = ctx.enter_context(tc.tile_pool(name="wpool", bufs=1))
            ppool = ctx.enter_context(tc.tile_pool(name="ppool", bufs=2))
            opool = ctx.enter_context(tc.tile_pool(name="opool", bufs=4))
            rpool = ctx.enter_context(tc.tile_pool(name="rpool", bufs=2))
            xpool = ctx.enter_context(tc.tile_pool(name="xpool", bufs=8))
            proj_ps = ctx.enter_context(
                tc.tile_pool(name="proj_ps", bufs=2, space="PSUM")
            )
            sc_ps = ctx.enter_context(tc.tile_pool(name="sc_ps", bufs=2, space="PSUM"))
            xt_ps = ctx.enter_context(tc.tile_pool(name="xt_ps", bufs=2, space="PSUM"))

            # --- SBUF resident tensors ---
            qT = big.tile([P, NCH, S], BF16, tag="qT")
            keyT = big.tile([P, NCH, S], BF16, tag="share1")  # reused later by xT
            valT = big.tile([P, NCH, S], BF16, tag="valT")
            kT = big.tile([P, NCH, S], BF16, tag="kT")
            vpad = big.tile([P, NCH, H, DK + 1], BF16, tag="vpad")
            wk = wpool.tile([P, NCH, D], BF16, tag="wk")
            wv = wpool.tile([P, NCH, D], BF16, tag="wv")
            wo = wpool.tile([P, NCH, D], BF16, tag="wo")
            bk_sb = const.tile([P, NCH], F32, tag="bk")
            bv_b = const.tile([P, D], F32, tag="bv")
            bo_b = const.tile([P, D], F32, tag="bo")

            # --- input DMAs (V-proj deps first: it must fully precede p@v) ---
            nc.sync.dma_start(out=valT[:], in_=valT_d[:].rearrange("c p f -> p c f"))
            nc.sync.dma_start(out=wv[:], in_=wvT_d[:].rearrange("c p f -> p c f"))
            nc.sync.dma_start(out=bv_b[:], in_=bv_d[:][None, :].to_broadcast((P, D)))
            nc.sync.dma_start(out=keyT[:], in_=keyT_d[:].rearrange("c p f -> p c f"))
            nc.sync.dma_start(out=wk[:], in_=wkT_d[:].rearrange("c p f -> p c f"))
            nc.sync.dma_start(out=bk_sb[:], in_=bk_d[:].rearrange("(c p) -> p c", p=P))
            nc.sync.dma_start(out=qT[:], in_=qT_d[:].rearrange("c p f -> p c f"))
            nc.sync.dma_start(out=wo[:], in_=woT_d[:].rearrange("c p f -> p c f"))
            nc.sync.dma_start(out=bo_b[:], in_=bo_d[:][None, :].to_broadcast((P, D)))

            # --- 1. K_T = Wk @ key.T + bk  (feature-major) ---
            # m-tile 0 runs in the prologue; tiles 1..7 are interleaved into
            # attention chunk 0 as PE filler (head pair m needs only tile m).
            def emit_ktproj(m):
                ps = [
                    proj_ps.tile([P, QC], F32, tag="pp", name=f"kp{m}_{j}")
                    for j in range(NQC)
                ]
                for c in range(NCH):  # d_in chunk (contraction)
                    st = wk[:, c, m * P : (m + 1) * P]
                    for j in range(NQC):
                        nc.tensor.matmul(
                            ps[j][:],
                            st,
                            keyT[:, c, j * QC : (j + 1) * QC],
                            start=(c == 0),
                            stop=(c == NCH - 1),
                        )
                for j in range(NQC):
                    nc.vector.tensor_scalar_add(
                        kT[:, m, j * QC : (j + 1) * QC], ps[j][:], bk_sb[:, m : m + 1]
                    )

            # --- 2. V = value @ Wv.T + bv (token-major, head-padded w/ ones) ---
            for t in range(NCH):  # s tile
                ps = [proj_ps.tile([P, QC], F32, tag="pp", name=f"pp{j}") for j in range(NQC)]
                for c in range(NCH):
                    st = valT[:, c, t * P : (t + 1) * P]
                    for j in range(NQC):
                        nc.tensor.matmul(
                            ps[j][:],
                            st,
                            wv[:, c, j * QC : (j + 1) * QC],
                            start=(c == 0),
                            stop=(c == NCH - 1),
                        )
                hpc = QC // DK  # heads per psum chunk
                for j in range(NQC):
                    nc.vector.tensor_tensor(
                        vpad[:, t, j * hpc : (j + 1) * hpc, 0:DK],
                        ps[j][:].rearrange("p (h d) -> p h d", d=DK),
                        bv_b[:, j * QC : (j + 1) * QC].rearrange(
                            "p (h d) -> p h d", d=DK
                        ),
                        op=ADD,
                    )
                nc.vector.memset(vpad[:, t, :, DK : DK + 1], 1.0)

            emit_ktproj(0)  # head pair 0's K_T; tiles 1..7 interleave below

            # --- 3+4. per-head attention, q-chunk-outer, head-pipelined ---
            # PE is in-order: emit scores(h) before p@v(h-1) so the PE has
            # work while ACT chews through exp(h-1). O-projection s-tiles
            # for q-chunk j unlock once all heads finished chunk j; they are
            # interleaved into the following chunk (extra PE filler).
            xT = big.tile([P, NCH, S], BF16, tag="share1")  # reuses keyT slot

            def emit_scores(h, j):
                """scoresT + exp for head h, q-chunk j -> pt tile (returned)."""
                ch, off = divmod(h, 2)
                off *= DK
                pt = ppool.tile([P, NCH, QC], BF16, tag="pt", name=f"pt{h}_{j}")
                for kp in range(NCH // 2):  # kt pairs share one 2-bank psum
                    sp = sc_ps.tile([P, 2, QC], F32, tag="sp", name=f"sp{h}{j}{kp}")
                    for u in range(2):
                        kt = 2 * kp + u
                        nc.tensor.matmul(
                            sp[:, u, :],
                            kT[off : off + DK, ch, kt * P : (kt + 1) * P],
                            qT[off : off + DK, ch, j * QC : (j + 1) * QC],
                            start=True,
                            stop=True,
                        )
                    nc.scalar.activation(
                        pt[:, 2 * kp : 2 * kp + 2, :], sp[:], EXP, scale=SCALE
                    )
                return pt

            def emit_pv(h, j, pt, dcols, xsb_map):
                """p@v for head h chunk j: unnormalized x -> SBUF, denom -> dcol.

                Engine APs need 32-aligned start partitions, so the 4 denoms
                of a batch land at partitions 0/32/64/96 of one collector."""
                xp = xt_ps.tile([DK + 1, QC], F32, tag="xp", name=f"xp{h}_{j}")
                for kc in range(NCH):
                    nc.tensor.matmul(
                        xp[:],
                        vpad[:, kc, h, :],
                        pt[:, kc, :],
                        start=(kc == 0),
                        stop=(kc == NCH - 1),
                    )
                xsb = xpool.tile([DK, QC], BF16, tag="xsb", name=f"xsb{h}_{j}")
                nc.vector.tensor_copy(xsb[:], xp[0:DK, :])
                b, r = divmod(h, 4)
                if r == 0:
                    dcols[b] = rpool.tile([97, QC], F32, tag="dcol", name=f"dc{j}_{b}")
                    nc.vector.memset(dcols[b][:], 1.0)  # only rows 0/32/64/96 matter
                nc.vector.tensor_copy(dcols[b][32 * r : 32 * r + 1, :], xp[DK : DK + 1, :])
                xsb_map[h] = xsb

            def emit_recip_half(j, hb, half, dcols, rcols):
                """half of the batch reciprocal (split so the DVE queue never
                blocks >~1.7us in front of the xp-slot-freeing copies)."""
                if half == 0:
                    rcols[hb // 4] = rpool.tile(
                        [97, QC], F32, tag="rcol", name=f"rc{j}_{hb}"
                    )
                sl = slice(half * (QC // 2), (half + 1) * (QC // 2))
                nc.vector.reciprocal(rcols[hb // 4][:, sl], dcols[hb // 4][:, sl])

            def emit_bcast(j, hh, rcols, rb_map):
                r = 32 * (hh % 4)
                # partition_broadcast ucode reads via Q7 core 0 -> input
                # must sit at base partition 0; bounce the row there.
                rb0 = rpool.tile([1, QC], F32, tag="rb0", name=f"rb0{j}_{hh}")
                nc.vector.tensor_copy(rb0[:], rcols[hh // 4][r : r + 1, :])
                rb = rpool.tile([DK, QC], F32, tag="rb", name=f"rb{j}_{hh}", bufs=6)
                nc.gpsimd.partition_broadcast(rb[:], rb0[:])
                rb_map[hh] = rb

            def emit_norm_mult(j, hh, xsb_map, rb_map):
                """deferred multiply (by now the broadcast is long done)."""
                ch, off = divmod(hh, 2)
                off *= DK
                nc.vector.tensor_tensor(
                    xT[off : off + DK, ch, j * QC : (j + 1) * QC],
                    xsb_map[hh][:],
                    rb_map[hh][:],
                    op=MULT,
                )

            def emit_oproj(t):
                """O = x @ Wo.T + bo for s-tile t (needs all of xT cols of t)."""
                ps = [
                    proj_ps.tile([P, QC], F32, tag="pp", name=f"op{t}_{j}")
                    for j in range(NQC)
                ]
                for c in range(NCH):
                    st = xT[:, c, t * P : (t + 1) * P]
                    for j in range(NQC):
                        nc.tensor.matmul(
                            ps[j][:],
                            st,
                            wo[:, c, j * QC : (j + 1) * QC],
                            start=(c == 0),
                            stop=(c == NCH - 1),
                        )
                for j in range(NQC):
                    ot = opool.tile([P, QC], F32, tag="ot", name=f"ot{t}_{j}")
                    nc.vector.tensor_tensor(
                        ot[:], ps[j][:], bo_b[:, j * QC : (j + 1) * QC], op=ADD
                    )
                    nc.sync.dma_start(
                        out=out_d[t * P : (t + 1) * P, j * QC : (j + 1) * QC],
                        in_=ot[:],
                    )

            for j in range(NQC):
                dcols = {}
                xsb_map = {}
                rcols = {}
                rb_map = {}
                prev = None  # (h, pt) pending p@v
                for h in range(H):
                    # PE filler: chunk 0 takes K-proj tiles, chunk 1 O-proj
                    if j == 0 and h % 2 == 1 and h < 15:
                        emit_ktproj((h + 1) // 2)
                    if j > 0 and h in (4, 6, 8, 10):
                        emit_oproj((h - 4) // 2)
                    pt = emit_scores(h, j)
                    if prev is not None:
                        emit_pv(prev[0], j, prev[1], dcols, xsb_map)
                    prev = (h, pt)
                    # spread the normalization chain of batch b=(h-4)//4 one
                    # small op-group per head slot, so no engine queue blocks
                    if h >= 4:
                        b4 = ((h - 4) // 4) * 4  # batch head base
                        r = (h - 4) % 4
                        if r == 0:
                            emit_recip_half(j, b4, 0, dcols, rcols)
                        elif r == 1:
                            emit_recip_half(j, b4, 1, dcols, rcols)
                            emit_bcast(j, b4, rcols, rb_map)
                            emit_bcast(j, b4 + 1, rcols, rb_map)
                        elif r == 2:
                            emit_bcast(j, b4 + 2, rcols, rb_map)
                            emit_bcast(j, b4 + 3, rcols, rb_map)
                            emit_norm_mult(j, b4, xsb_map, rb_map)
                            emit_norm_mult(j, b4 + 1, xsb_map, rb_map)
                        else:
                            emit_norm_mult(j, b4 + 2, xsb_map, rb_map)
                            emit_norm_mult(j, b4 + 3, xsb_map, rb_map)
                emit_pv(prev[0], j, prev[1], dcols, xsb_map)
                for half in range(2):
                    emit_recip_half(j, 12, half, dcols, rcols)
                for hh in range(12, H):
                    emit_bcast(j, hh, rcols, rb_map)
                for hh in range(12, H):
                    emit_norm_mult(j, hh, xsb_map, rb_map)
            # tail: O-proj s-tiles of the last q-chunk
            for t in range(4, NCH):
                emit_oproj(t)

    nc.finalize()
    return nc


def get_nc():
    if "nc" not in _CACHE:
        _CACHE["nc"] = _build_nc()
    return _CACHE["nc"]


def _tp_bf16(a):
    """[X, Y] f32 -> transposed bf16 [NCH, P, Y]."""
    return (
        np.ascontiguousarray(np.asarray(a, dtype=np.float32).T)
        .astype(ml_dtypes.bfloat16)
        .reshape(NCH, P, -1)
    )


def make_in_maps(query, key, value, Wk, bk, Wv, bv, Wo, bo):
    wkT = _tp_bf16(Wk)
    wvT = _tp_bf16(Wv)
    woT = _tp_bf16(Wo)
    bk = np.asarray(bk, dtype=np.float32)
    bv = np.asarray(bv, dtype=np.float32)
    bo = np.asarray(bo, dtype=np.float32)
    in_maps = []
    for b in range(B):
        in_maps.append(
            {
                "qT": _tp_bf16(query[b]),
                "keyT": _tp_bf16(key[b]),
                "valT": _tp_bf16(value[b]),
                "wkT": wkT,
                "wvT": wvT,
                "woT": woT,
                "bk": bk,
                "bv": bv,
                "bo": bo,
            }
        )
    return in_maps


def run(trace=False, **inputs):
    from concourse.bass_utils import run_bass_kernel_spmd

    nc = get_nc()
    in_maps = make_in_maps(**inputs)
    res = run_bass_kernel_spmd(nc, in_maps, list(range(N_CORES)), trace=trace)
    out = np.stack([res.results[i]["out"] for i in range(N_CORES)], axis=0)
    return out, res


def kernel(**inputs):
    out, _ = run(trace=False, **inputs)
    return out


# revision 37
# speedup vs baseline: 1.7008x; 1.0441x over previous
"""Multi-headed attention (B=8, S=1024, D=1024, H=16) on 8 TRN2 NeuronCores.

Strategy: pure data parallelism over the batch — core b computes batch element b
end-to-end (no collectives). All matmuls in bf16 (fp32 PSUM accumulation).

Per-core dataflow (everything "T" is feature-major [D, S]):
  inputs (host-pretransposed, bf16): qT, keyT, valT, wkT, wvT, woT
  1. K_T[d_out, s]  = matmul(lhsT=wkT, rhs=keyT) + bk      (bias per-partition)
  2. V[s, d_out]    = matmul(lhsT=valT, rhs=wvT) + bv      -> packed [s, h, 65]
                      with a ones column per head (gives softmax denominators
                      for free inside the p@v matmul)
  3. per head h: scoresT[k, q] = matmul(lhsT=K_T_h[64,128], rhs=qT_h[64,512])
                 pT = exp(scoresT / 8)  (ACT; max-subtraction skipped — scores
                 are provably small for this problem)
  4. xT_h[65, q] accum = matmul(lhsT=[V_h | 1][128,65], rhs=pT[128,512]);
     row 64 = softmax denominator; normalize rows 0..63 by its reciprocal
     (partition-broadcast via DMA)
  5. O[s, d_out] = matmul(lhsT=xT, rhs=woT) + bo -> DMA out (f32)
"""

import numpy as np
import ml_dtypes

import concourse.bass as bass
import concourse.bacc as bacc
import concourse.mybir as mybir
import concourse.tile as tile
from contextlib import ExitStack

B, S, D, H = 8, 1024, 1024, 16
P = 128
DK = D // H          # 64
NCH = D // P         # 8
QC = 512             # free-dim chunk (one PSUM bank)
NQC = S // QC        # 2
SCALE = 1.0 / float(np.sqrt(DK))  # 0.125
N_CORES = 8

BF16 = mybir.dt.bfloat16
F32 = mybir.dt.float32
ADD = mybir.AluOpType.add
MULT = mybir.AluOpType.mult
EXP = mybir.ActivationFunctionType.Exp

_CACHE = {}


def _build_nc():
    nc = bacc.Bacc(None)

    qT_d = nc.dram_tensor("qT", [NCH, P, S], BF16, kind="ExternalInput")
    keyT_d = nc.dram_tensor("keyT", [NCH, P, S], BF16, kind="ExternalInput")
    valT_d = nc.dram_tensor("valT", [NCH, P, S], BF16, kind="ExternalInput")
    wkT_d = nc.dram_tensor("wkT", [NCH, P, D], BF16, kind="ExternalInput")
    wvT_d = nc.dram_tensor("wvT", [NCH, P, D], BF16, kind="ExternalInput")
    woT_d = nc.dram_tensor("woT", [NCH, P, D], BF16, kind="ExternalInput")
    bk_d = nc.dram_tensor("bk", [D], F32, kind="ExternalInput")
    bv_d = nc.dram_tensor("bv", [D], F32, kind="ExternalInput")
    bo_d = nc.dram_tensor("bo", [D], F32, kind="ExternalInput")
    out_d = nc.dram_tensor("out", [S, D], F32, kind="ExternalOutput")

    with tile.TileContext(nc) as tc:
        with ExitStack() as ctx:
            const = ctx.enter_context(tc.tile_pool(name="const", bufs=1))
            big = ctx.enter_context(tc.tile_pool(name="big", bufs=1))
            wpool = ctx.enter_context(tc.tile_pool(name="wpool", bufs=1))
            ppool = ctx.enter_context(tc.tile_pool(name="ppool", bufs=2))
            opool = ctx.enter_context(tc.tile_pool(name="opool", bufs=4))
            rpool = ctx.enter_context(tc.tile_pool(name="rpool", bufs=2))
            xpool = ctx.enter_context(tc.tile_pool(name="xpool", bufs=10))
            proj_ps = ctx.enter_context(
                tc.tile_pool(name="proj_ps", bufs=2, space="PSUM")
            )
            sc_ps = ctx.enter_context(tc.tile_pool(name="sc_ps", bufs=2, space="PSUM"))
            xt_ps = ctx.enter_context(tc.tile_pool(name="xt_ps", bufs=2, space="PSUM"))

            # --- SBUF resident tensors ---
            qT = big.tile([P, NCH, S], BF16, tag="qT")
            keyT = big.tile([P, NCH, S], BF16, tag="share1")  # reused later by xT
            valT = big.tile([P, NCH, S], BF16, tag="valT")
            kT = big.tile([P, NCH, S], BF16, tag="kT")
            vpad = big.tile([P, NCH, H, DK + 1], BF16, tag="vpad")
            wk = wpool.tile([P, NCH, D], BF16, tag="wk")
            wv = wpool.tile([P, NCH, D], BF16, tag="wv")
            wo = wpool.tile([P, NCH, D], BF16, tag="wo")
            bk_sb = const.tile([P, NCH], F32, tag="bk")
            bv_b = const.tile([P, D], F32, tag="bv")
            bo_b = const.tile([P, D], F32, tag="bo")

            # --- input DMAs (V-proj deps first: it must fully precede p@v) ---
            nc.sync.dma_start(out=valT[:], in_=valT_d[:].rearrange("c p f -> p c f"))
            nc.sync.dma_start(out=wv[:], in_=wvT_d[:].rearrange("c p f -> p c f"))
            nc.sync.dma_start(out=bv_b[:], in_=bv_d[:][None, :].to_broadcast((P, D)))
            nc.sync.dma_start(out=keyT[:], in_=keyT_d[:].rearrange("c p f -> p c f"))
            nc.sync.dma_start(out=wk[:], in_=wkT_d[:].rearrange("c p f -> p c f"))
            nc.sync.dma_start(out=bk_sb[:], in_=bk_d[:].rearrange("(c p) -> p c", p=P))
            nc.sync.dma_start(out=qT[:], in_=qT_d[:].rearrange("c p f -> p c f"))
            nc.sync.dma_start(out=wo[:], in_=woT_d[:].rearrange("c p f -> p c f"))
            nc.sync.dma_start(out=bo_b[:], in_=bo_d[:][None, :].to_broadcast((P, D)))

            # --- 1. K_T = Wk @ key.T + bk  (feature-major) ---
            # m-tile 0 runs in the prologue; tiles 1..7 are interleaved into
            # attention chunk 0 as PE filler (head pair m needs only tile m).
            def emit_ktproj(m):
                ps = [
                    proj_ps.tile([P, QC], F32, tag="pp", name=f"kp{m}_{j}")
                    for j in range(NQC)
                ]
                for c in range(NCH):  # d_in chunk (contraction)
                    st = wk[:, c, m * P : (m + 1) * P]
                    for j in range(NQC):
                        nc.tensor.matmul(
                            ps[j][:],
                            st,
                            keyT[:, c, j * QC : (j + 1) * QC],
                            start=(c == 0),
                            stop=(c == NCH - 1),
                        )
                for j in range(NQC):
                    nc.vector.tensor_scalar_add(
                        kT[:, m, j * QC : (j + 1) * QC], ps[j][:], bk_sb[:, m : m + 1]
                    )

            # --- 2. V = value @ Wv.T + bv (token-major, head-padded w/ ones) ---
            for t in range(NCH):  # s tile
                ps = [proj_ps.tile([P, QC], F32, tag="pp", name=f"pp{j}") for j in range(NQC)]
                for c in range(NCH):
                    st = valT[:, c, t * P : (t + 1) * P]
                    for j in range(NQC):
                        nc.tensor.matmul(
                            ps[j][:],
                            st,
                            wv[:, c, j * QC : (j + 1) * QC],
                            start=(c == 0),
                            stop=(c == NCH - 1),
                        )
                hpc = QC // DK  # heads per psum chunk
                for j in range(NQC):
                    nc.vector.tensor_tensor(
                        vpad[:, t, j * hpc : (j + 1) * hpc, 0:DK],
                        ps[j][:].rearrange("p (h d) -> p h d", d=DK),
                        bv_b[:, j * QC : (j + 1) * QC].rearrange(
                            "p (h d) -> p h d", d=DK
                        ),
                        op=ADD,
                    )
                nc.vector.memset(vpad[:, t, :, DK : DK + 1], 1.0)

            emit_ktproj(0)  # head pair 0's K_T; tiles 1..7 interleave below

            # --- 3+4. per-head attention, q-chunk-outer, head-pipelined ---
            # PE is in-order: emit scores(h) before p@v(h-1) so the PE has
            # work while ACT chews through exp(h-1). O-projection s-tiles
            # for q-chunk j unlock once all heads finished chunk j; they are
            # interleaved into the following chunk (extra PE filler).
            xT = big.tile([P, NCH, S], BF16, tag="share1")  # reuses keyT slot

            def emit_scores(h, j):
                """scoresT + exp for head h, q-chunk j -> pt tile (returned)."""
                ch, off = divmod(h, 2)
                off *= DK
                pt = ppool.tile([P, NCH, QC], BF16, tag="pt", name=f"pt{h}_{j}")
                for kp in range(NCH // 2):  # kt pairs share one 2-bank psum
                    sp = sc_ps.tile([P, 2, QC], F32, tag="sp", name=f"sp{h}{j}{kp}")
                    for u in range(2):
                        kt = 2 * kp + u
                        nc.tensor.matmul(
                            sp[:, u, :],
                            kT[off : off + DK, ch, kt * P : (kt + 1) * P],
                            qT[off : off + DK, ch, j * QC : (j + 1) * QC],
                            start=True,
                            stop=True,
                        )
                    nc.scalar.activation(
                        pt[:, 2 * kp : 2 * kp + 2, :], sp[:], EXP, scale=SCALE
                    )
                return pt

            def emit_pv(p, pt, dcols, xsb_map):
                """p@v for global position p: unnormalized x -> SBUF, denom -> dcol.

                Engine APs need 32-aligned start partitions, so the 4 denoms
                of a batch land at partitions 0/32/64/96 of one collector."""
                j, h = divmod(p, H)
                xp = xt_ps.tile([DK + 1, QC], F32, tag="xp", name=f"xp{p}")
                for kc in range(NCH):
                    nc.tensor.matmul(
                        xp[:],
                        vpad[:, kc, h, :],
                        pt[:, kc, :],
                        start=(kc == 0),
                        stop=(kc == NCH - 1),
                    )
                xsb = xpool.tile([DK, QC], BF16, tag="xsb", name=f"xsb{p}")
                nc.vector.tensor_copy(xsb[:], xp[0:DK, :])
                b, r = divmod(p, 4)
                if r == 0:
                    dcols[b] = rpool.tile([97, QC], F32, tag="dcol", name=f"dc{b}")
                    nc.vector.memset(dcols[b][:], 1.0)  # only rows 0/32/64/96 matter
                nc.vector.tensor_copy(dcols[b][32 * r : 32 * r + 1, :], xp[DK : DK + 1, :])
                xsb_map[p] = xsb

            def emit_recip_half(j, pb, half, dcols, rcols):
                """half of the batch reciprocal (split so the DVE queue never
                blocks >~1.7us in front of the xp-slot-freeing copies)."""
                if half == 0:
                    rcols[pb // 4] = rpool.tile([97, QC], F32, tag="rcol", name=f"rc{pb}")
                sl = slice(half * (QC // 2), (half + 1) * (QC // 2))
                nc.vector.reciprocal(rcols[pb // 4][:, sl], dcols[pb // 4][:, sl])

            def emit_bcast(j, p, rcols, rb_map):
                r = 32 * (p % 4)
                # partition_broadcast ucode reads via Q7 core 0 -> input
                # must sit at base partition 0; bounce the row there.
                rb0 = rpool.tile([1, QC], F32, tag="rb0", name=f"rb0_{p}")
                nc.vector.tensor_copy(rb0[:], rcols[p // 4][r : r + 1, :])
                rb = rpool.tile([DK, QC], F32, tag="rb", name=f"rb{p}", bufs=6)
                nc.gpsimd.partition_broadcast(rb[:], rb0[:])
                rb_map[p] = rb

            def emit_norm_mult(j, p, xsb_map, rb_map):
                """deferred multiply (by now the broadcast is long done)."""
                ch, off = divmod(p % H, 2)
                off *= DK
                nc.vector.tensor_tensor(
                    xT[off : off + DK, ch, j * QC : (j + 1) * QC],
                    xsb_map[p][:],
                    rb_map[p][:],
                    op=MULT,
                )

            def emit_oproj(t):
                """O = x @ Wo.T + bo for s-tile t (needs all of xT cols of t)."""
                ps = [
                    proj_ps.tile([P, QC], F32, tag="pp", name=f"op{t}_{j}")
                    for j in range(NQC)
                ]
                for c in range(NCH):
                    st = xT[:, c, t * P : (t + 1) * P]
                    for j in range(NQC):
                        nc.tensor.matmul(
                            ps[j][:],
                            st,
                            wo[:, c, j * QC : (j + 1) * QC],
                            start=(c == 0),
                            stop=(c == NCH - 1),
                        )
                for j in range(NQC):
                    ot = opool.tile([P, QC], F32, tag="ot", name=f"ot{t}_{j}")
                    nc.vector.tensor_tensor(
                        ot[:], ps[j][:], bo_b[:, j * QC : (j + 1) * QC], op=ADD
                    )
                    nc.sync.dma_start(
                        out=out_d[t * P : (t + 1) * P, j * QC : (j + 1) * QC],
                        in_=ot[:],
                    )

            # ---- flat 32-step pipeline over (chunk, head) ----
            # position p = 16*j + h. Norm chain of each 4-head batch is
            # spread one small op-group per later step (crossing chunk
            # boundaries) so no engine queue ever blocks the PE for long.
            dcols = {}
            xsb_map = {}
            rcols = {}
            rb_map = {}
            NPOS = NQC * H

            def norm_step(s):
                for p, acts in (
                    (s - 4, "r0"),
                    (s - 5, "r1"),
                    (s - 6, "b01"),
                    (s - 7, "b23m0"),
                    (s - 8, "m12"),
                    (s - 9, "m3"),
                ):
                    if p < 0 or p % 4 != 0 or p >= NPOS:
                        continue
                    j = p // H
                    if acts == "r0":
                        emit_recip_half(j, p, 0, dcols, rcols)
                    elif acts == "r1":
                        emit_recip_half(j, p, 1, dcols, rcols)
                    elif acts == "b01":
                        emit_bcast(j, p, rcols, rb_map)
                        emit_bcast(j, p + 1, rcols, rb_map)
                    elif acts == "b23m0":
                        emit_bcast(j, p + 2, rcols, rb_map)
                        emit_bcast(j, p + 3, rcols, rb_map)
                        emit_norm_mult(j, p, xsb_map, rb_map)
                    elif acts == "m12":
                        emit_norm_mult(j, p + 1, xsb_map, rb_map)
                        emit_norm_mult(j, p + 2, xsb_map, rb_map)
                    else:
                        emit_norm_mult(j, p + 3, xsb_map, rb_map)

            prev = None  # (pos, pt) pending p@v
            for s in range(NPOS):
                j, h = divmod(s, H)
                # PE filler: chunk 0 takes K-proj tiles; chunk 1 takes the
                # O-proj tiles that depend only on chunk-0 columns.
                if j == 0 and h % 2 == 1 and h < 15:
                    emit_ktproj((h + 1) // 2)
                if j == 1 and h in (7, 9, 11):
                    emit_oproj((h - 7) // 2)
                pt = emit_scores(h, j)
                if prev is not None:
                    emit_pv(prev[0], prev[1], dcols, xsb_map)
                prev = (s, pt)
                norm_step(s)
            emit_pv(prev[0], prev[1], dcols, xsb_map)
            emit_oproj(3)  # chunk-0-dependent tile: PE work during last chain
            for s in range(NPOS, NPOS + 6):
                norm_step(s)
            # tail: O-proj s-tiles of the last q-chunk
            for t in range(4, NCH):
                emit_oproj(t)

    nc.finalize()
    return nc


def get_nc():
    if "nc" not in _CACHE:
        _CACHE["nc"] = _build_nc()
    return _CACHE["nc"]


def _tp_bf16(a):
    """[X, Y] f32 -> transposed bf16 [NCH, P, Y]."""
    return (
        np.ascontiguousarray(np.asarray(a, dtype=np.float32).T)
        .astype(ml_dtypes.bfloat16)
        .reshape(NCH, P, -1)
    )


def make_in_maps(query, key, value, Wk, bk, Wv, bv, Wo, bo):
    wkT = _tp_bf16(Wk)
    wvT = _tp_bf16(Wv)
    woT = _tp_bf16(Wo)
    bk = np.asarray(bk, dtype=np.float32)
    bv = np.asarray(bv, dtype=np.float32)
    bo = np.asarray(bo, dtype=np.float32)
    in_maps = []
    for b in range(B):
        in_maps.append(
            {
                "qT": _tp_bf16(query[b]),
                "keyT": _tp_bf16(key[b]),
                "valT": _tp_bf16(value[b]),
                "wkT": wkT,
                "wvT": wvT,
                "woT": woT,
                "bk": bk,
                "bv": bv,
                "bo": bo,
            }
        )
    return in_maps


def run(trace=False, **inputs):
    from concourse.bass_utils import run_bass_kernel_spmd

    nc = get_nc()
    in_maps = make_in_maps(**inputs)
    res = run_bass_kernel_spmd(nc, in_maps, list(range(N_CORES)), trace=trace)
    out = np.stack([res.results[i]["out"] for i in range(N_CORES)], axis=0)
    return out, res


def kernel(**inputs):
    out, _ = run(trace=False, **inputs)
    return out


# revision 40
# speedup vs baseline: 1.7253x; 1.0144x over previous
"""Multi-headed attention (B=8, S=1024, D=1024, H=16) on 8 TRN2 NeuronCores.

Strategy: pure data parallelism over the batch — core b computes batch element b
end-to-end (no collectives). All matmuls in bf16 (fp32 PSUM accumulation).

Per-core dataflow (everything "T" is feature-major [D, S]):
  inputs (host-pretransposed, bf16): qT, keyT, valT, wkT, wvT, woT
  1. K_T[d_out, s]  = matmul(lhsT=wkT, rhs=keyT) + bk      (bias per-partition)
  2. V[s, d_out]    = matmul(lhsT=valT, rhs=wvT) + bv      -> packed [s, h, 65]
                      with a ones column per head (gives softmax denominators
                      for free inside the p@v matmul)
  3. per head h: scoresT[k, q] = matmul(lhsT=K_T_h[64,128], rhs=qT_h[64,512])
                 pT = exp(scoresT / 8)  (ACT; max-subtraction skipped — scores
                 are provably small for this problem)
  4. xT_h[65, q] accum = matmul(lhsT=[V_h | 1][128,65], rhs=pT[128,512]);
     row 64 = softmax denominator; normalize rows 0..63 by its reciprocal
     (partition-broadcast via DMA)
  5. O[s, d_out] = matmul(lhsT=xT, rhs=woT) + bo -> DMA out (f32)
"""

import numpy as np
import ml_dtypes

import concourse.bass as bass
import concourse.bacc as bacc
import concourse.mybir as mybir
import concourse.tile as tile
from contextlib import ExitStack

B, S, D, H = 8, 1024, 1024, 16
P = 128
DK = D // H          # 64
NCH = D // P         # 8
QC = 512             # free-dim chunk (one PSUM bank)
NQC = S // QC        # 2
SCALE = 1.0 / float(np.sqrt(DK))  # 0.125
N_CORES = 8

BF16 = mybir.dt.bfloat16
F32 = mybir.dt.float32
ADD = mybir.AluOpType.add
MULT = mybir.AluOpType.mult
EXP = mybir.ActivationFunctionType.Exp

_CACHE = {}


def _build_nc():
    nc = bacc.Bacc(None)

    qT_d = nc.dram_tensor("qT", [NCH, P, S], BF16, kind="ExternalInput")
    keyT_d = nc.dram_tensor("keyT", [NCH, P, S], BF16, kind="ExternalInput")
    valT_d = nc.dram_tensor("valT", [NCH, P, S], BF16, kind="ExternalInput")
    wkT_d = nc.dram_tensor("wkT", [NCH, P, D], BF16, kind="ExternalInput")
    wvT_d = nc.dram_tensor("wvT", [NCH, P, D], BF16, kind="ExternalInput")
    woT_d = nc.dram_tensor("woT", [NCH, P, D], BF16, kind="ExternalInput")
    bk_d = nc.dram_tensor("bk", [D], F32, kind="ExternalInput")
    bv_d = nc.dram_tensor("bv", [D], F32, kind="ExternalInput")
    bo_d = nc.dram_tensor("bo", [D], F32, kind="ExternalInput")
    out_d = nc.dram_tensor("out", [S, D], F32, kind="ExternalOutput")

    with tile.TileContext(nc) as tc:
        with ExitStack() as ctx:
            const = ctx.enter_context(tc.tile_pool(name="const", bufs=1))
            big = ctx.enter_context(tc.tile_pool(name="big", bufs=1))
            wpool = ctx.enter_context(tc.tile_pool(name="wpool", bufs=1))
            ppool = ctx.enter_context(tc.tile_pool(name="ppool", bufs=2))
            opool = ctx.enter_context(tc.tile_pool(name="opool", bufs=4))
            rpool = ctx.enter_context(tc.tile_pool(name="rpool", bufs=2))
            xpool = ctx.enter_context(tc.tile_pool(name="xpool", bufs=10))
            proj_ps = ctx.enter_context(
                tc.tile_pool(name="proj_ps", bufs=2, space="PSUM")
            )
            sc_ps = ctx.enter_context(tc.tile_pool(name="sc_ps", bufs=2, space="PSUM"))
            xt_ps = ctx.enter_context(tc.tile_pool(name="xt_ps", bufs=2, space="PSUM"))

            # --- SBUF resident tensors ---
            qT = big.tile([P, NCH, S], BF16, tag="qT")
            keyT = big.tile([P, NCH, S], BF16, tag="share1")  # reused later by xT
            valT = big.tile([P, NCH, S], BF16, tag="valT")
            kT = big.tile([P, NCH, S], BF16, tag="kT")
            vpad = big.tile([P, NCH, H, DK + 1], BF16, tag="vpad")
            wk = wpool.tile([P, NCH, D], BF16, tag="wk")
            wv = wpool.tile([P, NCH, D], BF16, tag="wv")
            wo = wpool.tile([P, NCH, D], BF16, tag="wo")
            bk_sb = const.tile([P, NCH], F32, tag="bk")
            bv_b = const.tile([P, D], F32, tag="bv")
            bo_b = const.tile([P, D], F32, tag="bo")

            # --- input DMAs (V-proj deps first: it must fully precede p@v) ---
            nc.sync.dma_start(out=valT[:], in_=valT_d[:].rearrange("c p f -> p c f"))
            nc.sync.dma_start(
                out=wv[:, :, 0:QC], in_=wvT_d[:, :, 0:QC].rearrange("c p f -> p c f")
            )
            nc.sync.dma_start(
                out=wv[:, :, QC:D], in_=wvT_d[:, :, QC:D].rearrange("c p f -> p c f")
            )
            nc.sync.dma_start(out=bv_b[:], in_=bv_d[:][None, :].to_broadcast((P, D)))
            nc.sync.dma_start(out=keyT[:], in_=keyT_d[:].rearrange("c p f -> p c f"))
            nc.sync.dma_start(out=wk[:], in_=wkT_d[:].rearrange("c p f -> p c f"))
            nc.sync.dma_start(out=bk_sb[:], in_=bk_d[:].rearrange("(c p) -> p c", p=P))
            nc.sync.dma_start(out=qT[:], in_=qT_d[:].rearrange("c p f -> p c f"))
            nc.sync.dma_start(out=wo[:], in_=woT_d[:].rearrange("c p f -> p c f"))
            nc.sync.dma_start(out=bo_b[:], in_=bo_d[:][None, :].to_broadcast((P, D)))

            # --- 1. K_T = Wk @ key.T + bk  (feature-major) ---
            # m-tile 0 runs in the prologue; tiles 1..7 are interleaved into
            # attention chunk 0 as PE filler (head pair m needs only tile m).
            def emit_ktproj(m):
                ps = [
                    proj_ps.tile([P, QC], F32, tag="pp", name=f"kp{m}_{j}")
                    for j in range(NQC)
                ]
                for c in range(NCH):  # d_in chunk (contraction)
                    st = wk[:, c, m * P : (m + 1) * P]
                    for j in range(NQC):
                        nc.tensor.matmul(
                            ps[j][:],
                            st,
                            keyT[:, c, j * QC : (j + 1) * QC],
                            start=(c == 0),
                            stop=(c == NCH - 1),
                        )
                for j in range(NQC):
                    nc.vector.tensor_scalar_add(
                        kT[:, m, j * QC : (j + 1) * QC], ps[j][:], bk_sb[:, m : m + 1]
                    )

            # --- 2. V = value @ Wv.T + bv (token-major, head-padded w/ ones) ---
            for t in range(NCH):  # s tile
                ps = [proj_ps.tile([P, QC], F32, tag="pp", name=f"pp{j}") for j in range(NQC)]
                for c in range(NCH):
                    st = valT[:, c, t * P : (t + 1) * P]
                    for j in range(NQC):
                        nc.tensor.matmul(
                            ps[j][:],
                            st,
                            wv[:, c, j * QC : (j + 1) * QC],
                            start=(c == 0),
                            stop=(c == NCH - 1),
                        )
                hpc = QC // DK  # heads per psum chunk
                for j in range(NQC):
                    nc.vector.tensor_tensor(
                        vpad[:, t, j * hpc : (j + 1) * hpc, 0:DK],
                        ps[j][:].rearrange("p (h d) -> p h d", d=DK),
                        bv_b[:, j * QC : (j + 1) * QC].rearrange(
                            "p (h d) -> p h d", d=DK
                        ),
                        op=ADD,
                    )
                nc.vector.memset(vpad[:, t, :, DK : DK + 1], 1.0)

            emit_ktproj(0)  # head pair 0's K_T; tiles 1..7 interleave below

            # --- 3+4. per-head attention, q-chunk-outer, head-pipelined ---
            # PE is in-order: emit scores(h) before p@v(h-1) so the PE has
            # work while ACT chews through exp(h-1). O-projection s-tiles
            # for q-chunk j unlock once all heads finished chunk j; they are
            # interleaved into the following chunk (extra PE filler).
            xT = big.tile([P, NCH, S], BF16, tag="share1")  # reuses keyT slot

            def emit_scores(h, j):
                """scoresT + exp for head h, q-chunk j -> pt tile (returned)."""
                ch, off = divmod(h, 2)
                off *= DK
                pt = ppool.tile([P, NCH, QC], BF16, tag="pt", name=f"pt{h}_{j}")
                for kp in range(NCH // 2):  # kt pairs share one 2-bank psum
                    sp = sc_ps.tile([P, 2, QC], F32, tag="sp", name=f"sp{h}{j}{kp}")
                    for u in range(2):
                        kt = 2 * kp + u
                        nc.tensor.matmul(
                            sp[:, u, :],
                            kT[off : off + DK, ch, kt * P : (kt + 1) * P],
                            qT[off : off + DK, ch, j * QC : (j + 1) * QC],
                            start=True,
                            stop=True,
                        )
                    nc.scalar.activation(
                        pt[:, 2 * kp : 2 * kp + 2, :], sp[:], EXP, scale=SCALE
                    )
                return pt

            def emit_pv(p, pt, dcols, xsb_map):
                """p@v for global position p: unnormalized x -> SBUF, denom -> dcol.

                Engine APs need 32-aligned start partitions, so the 4 denoms
                of a batch land at partitions 0/32/64/96 of one collector."""
                j, h = divmod(p, H)
                xp = xt_ps.tile([DK + 1, QC], F32, tag="xp", name=f"xp{p}")
                for kc in range(NCH):
                    nc.tensor.matmul(
                        xp[:],
                        vpad[:, kc, h, :],
                        pt[:, kc, :],
                        start=(kc == 0),
                        stop=(kc == NCH - 1),
                    )
                xsb = xpool.tile([DK, QC], BF16, tag="xsb", name=f"xsb{p}")
                nc.vector.tensor_copy(xsb[:], xp[0:DK, :])
                b, r = divmod(p, 4)
                if r == 0:
                    dcols[b] = rpool.tile([97, QC], F32, tag="dcol", name=f"dc{b}")
                    nc.vector.memset(dcols[b][:], 1.0)  # only rows 0/32/64/96 matter
                nc.vector.tensor_copy(dcols[b][32 * r : 32 * r + 1, :], xp[DK : DK + 1, :])
                xsb_map[p] = xsb

            def emit_recip_half(j, pb, half, dcols, rcols):
                """half of the batch reciprocal (split so the DVE queue never
                blocks >~1.7us in front of the xp-slot-freeing copies)."""
                if half == 0:
                    rcols[pb // 4] = rpool.tile([97, QC], F32, tag="rcol", name=f"rc{pb}")
                sl = slice(half * (QC // 2), (half + 1) * (QC // 2))
                nc.vector.reciprocal(rcols[pb // 4][:, sl], dcols[pb // 4][:, sl])

            def emit_bcast(j, p, rcols, rb_map):
                r = 32 * (p % 4)
                # partition_broadcast ucode reads via Q7 core 0 -> input
                # must sit at base partition 0; bounce the row there.
                rb0 = rpool.tile([1, QC], F32, tag="rb0", name=f"rb0_{p}")
                nc.vector.tensor_copy(rb0[:], rcols[p // 4][r : r + 1, :])
                rb = rpool.tile([DK, QC], F32, tag="rb", name=f"rb{p}", bufs=6)
                nc.gpsimd.partition_broadcast(rb[:], rb0[:])
                rb_map[p] = rb

            def emit_norm_mult(j, p, xsb_map, rb_map):
                """deferred multiply (by now the broadcast is long done)."""
                ch, off = divmod(p % H, 2)
                off *= DK
                nc.vector.tensor_tensor(
                    xT[off : off + DK, ch, j * QC : (j + 1) * QC],
                    xsb_map[p][:],
                    rb_map[p][:],
                    op=MULT,
                )

            def emit_oproj(t):
                """O = x @ Wo.T + bo for s-tile t (needs all of xT cols of t)."""
                ps = [
                    proj_ps.tile([P, QC], F32, tag="pp", name=f"op{t}_{j}")
                    for j in range(NQC)
                ]
                for c in range(NCH):
                    st = xT[:, c, t * P : (t + 1) * P]
                    for j in range(NQC):
                        nc.tensor.matmul(
                            ps[j][:],
                            st,
                            wo[:, c, j * QC : (j + 1) * QC],
                            start=(c == 0),
                            stop=(c == NCH - 1),
                        )
                for j in range(NQC):
                    ot = opool.tile([P, QC], F32, tag="ot", name=f"ot{t}_{j}")
                    nc.vector.tensor_tensor(
                        ot[:], ps[j][:], bo_b[:, j * QC : (j + 1) * QC], op=ADD
                    )
                    nc.sync.dma_start(
                        out=out_d[t * P : (t + 1) * P, j * QC : (j + 1) * QC],
                        in_=ot[:],
                    )

            # ---- flat 32-step pipeline over (chunk, head) ----
            # position p = 16*j + h. Norm chain of each 4-head batch is
            # spread one small op-group per later step (crossing chunk
            # boundaries) so no engine queue ever blocks the PE for long.
            dcols = {}
            xsb_map = {}
            rcols = {}
            rb_map = {}
            NPOS = NQC * H

            def norm_step(s):
                for p, acts in (
                    (s - 4, "r0"),
                    (s - 5, "r1"),
                    (s - 6, "b01"),
                    (s - 7, "b23m0"),
                    (s - 8, "m12"),
                    (s - 9, "m3"),
                ):
                    if p < 0 or p % 4 != 0 or p >= NPOS:
                        continue
                    j = p // H
                    if acts == "r0":
                        emit_recip_half(j, p, 0, dcols, rcols)
                    elif acts == "r1":
                        emit_recip_half(j, p, 1, dcols, rcols)
                    elif acts == "b01":
                        emit_bcast(j, p, rcols, rb_map)
                        emit_bcast(j, p + 1, rcols, rb_map)
                    elif acts == "b23m0":
                        emit_bcast(j, p + 2, rcols, rb_map)
                        emit_bcast(j, p + 3, rcols, rb_map)
                        emit_norm_mult(j, p, xsb_map, rb_map)
                    elif acts == "m12":
                        emit_norm_mult(j, p + 1, xsb_map, rb_map)
                        emit_norm_mult(j, p + 2, xsb_map, rb_map)
                    else:
                        emit_norm_mult(j, p + 3, xsb_map, rb_map)

            prev = None  # (pos, pt) pending p@v
            for s in range(NPOS):
                j, h = divmod(s, H)
                # PE filler: chunk 0 takes K-proj tiles; chunk 1 takes the
                # O-proj tiles that depend only on chunk-0 columns.
                if j == 0 and h % 2 == 1 and h < 15:
                    emit_ktproj((h + 1) // 2)
                if j == 1 and h in (7, 9):
                    emit_oproj((h - 7) // 2)
                pt = emit_scores(h, j)
                if prev is not None:
                    emit_pv(prev[0], prev[1], dcols, xsb_map)
                prev = (s, pt)
                norm_step(s)
            emit_pv(prev[0], prev[1], dcols, xsb_map)
            # chunk-0-dependent tiles held back: PE work covering last chain
            emit_oproj(2)
            emit_oproj(3)
            for s in range(NPOS, NPOS + 6):
                norm_step(s)
            # tail: O-proj s-tiles of the last q-chunk
            for t in range(4, NCH):
                emit_oproj(t)

    nc.finalize()
    return nc


def get_nc():
    if "nc" not in _CACHE:
        _CACHE["nc"] = _build_nc()
    return _CACHE["nc"]


def _tp_bf16(a):
    """[X, Y] f32 -> transposed bf16 [NCH, P, Y]."""
    return (
        np.ascontiguousarray(np.asarray(a, dtype=np.float32).T)
        .astype(ml_dtypes.bfloat16)
        .reshape(NCH, P, -1)
    )


def make_in_maps(query, key, value, Wk, bk, Wv, bv, Wo, bo):
    wkT = _tp_bf16(Wk)
    wvT = _tp_bf16(Wv)
    woT = _tp_bf16(Wo)
    bk = np.asarray(bk, dtype=np.float32)
    bv = np.asarray(bv, dtype=np.float32)
    bo = np.asarray(bo, dtype=np.float32)
    in_maps = []
    for b in range(B):
        in_maps.append(
            {
                "qT": _tp_bf16(query[b]),
                "keyT": _tp_bf16(key[b]),
                "valT": _tp_bf16(value[b]),
                "wkT": wkT,
                "wvT": wvT,
                "woT": woT,
                "bk": bk,
                "bv": bv,
                "bo": bo,
            }
        )
    return in_maps


def run(trace=False, **inputs):
    from concourse.bass_utils import run_bass_kernel_spmd

    nc = get_nc()
    in_maps = make_in_maps(**inputs)
    res = run_bass_kernel_spmd(nc, in_maps, list(range(N_CORES)), trace=trace)
    out = np.stack([res.results[i]["out"] for i in range(N_CORES)], axis=0)
    return out, res


def kernel(**inputs):
    out, _ = run(trace=False, **inputs)
    return out
